# revision 1
# baseline (speedup 1.0000x reference)
"""DiffAttention Trainium2 kernel.

Full inputs in, full output out. Sharding: 8 cores = (batch b in {0,1}) x
(head-pair p in {0..3}); each core handles one batch element and 2 of the 8
heads (= 4 of the 16 q/k half-heads, 2 v heads, 256 of the 1024 o columns).
Out-projection is column-split: each core produces a full (S, D) partial of
o @ Wo.T restricted to its o columns; host sums the 4 partials per batch.

All device matmuls run in fp16 (1 cycle/row on PE, fp32 PSUM accumulation).
Host pre-transposes operands so every matmul operand is loaded with natural
(contiguous) DMA:
  xT  = x[b].T                  (D, S)   rhs / lhsT for projections
  wqT = (Wq[rows].T) * hd^-0.5  (D, 256) lhsT for q^T projection (scaling folded)
  wkT = Wk[rows].T              (D, 256)
  wvT = Wv[rows].T              (D, 256) rhs for v projection
  woT = Wo[:, cols].T           (256, D) lhsT for out^T projection
Device returns outT_partial (D, S) fp16; host sums 4 partials per batch in
fp32 and transposes back.

Attention math per head h (half-heads e0=2h, e1=2h+1), per q row:
  u_i = exp(s_i) @ v   (unnormalized), sum_i = exp(s_i) @ 1  (fused: rhs=[v|1])
  o   = u0/sum0 - lam * u1/sum1
  o   = o * rsqrt(mean(o^2)+eps) * (1-lam_init);   out = o @ Wo.T
Scores are computed transposed (keys on partitions, q on free dim) so the
exp'd tiles feed the PV matmul directly as the stationary operand. rsqrt is
Newton-Raphson on the DVE (fast-inverse-sqrt seed), batched per (strip, head),
keeping the ACT engine exp-only (single activation table, no reload churn);
the (1-lam_init) factor is folded into the rsqrt argument. The main loop is
strip-major (512 q columns) so each strip's out-projection and output DMA
overlap the next strip's attention; strips run in order [0,3,1,2] (ending on
a mid-sized strip empirically minimizes the end-of-kernel drain). PSUM banks:
scores 2x2 (both half-heads
share one 1024-wide tile, exp'd by a single strided ACT op), u 2, and 2
shared by the o^T transposes and the out-projection.
"""

import math

import numpy as np

B = 2
S = 2048
D = 1024
H = 8
HD = 64  # half-head dim
LAMBDA_INIT = 0.8 - 0.6 * math.exp(-0.3 * 6)
EPS = 1e-5

N_CORES = 8
KT = D // 128      # 8 contraction tiles for projections
ST = S // 128      # 16 sequence tiles
NSTRIP = S // 512  # 4 q strips


def _build_program(lam: float, dbg: bool = False):
    import concourse.bass as bass
    import concourse.tile as tile
    from concourse import bacc, mybir
    from concourse.masks import make_identity

    f16 = mybir.dt.float16
    f32 = mybir.dt.float32
    u32 = mybir.dt.uint32
    AF = mybir.ActivationFunctionType
    OP = mybir.AluOpType

    nc = bacc.Bacc("TRN2", target_bir_lowering=False, debug=False,
                   num_devices=N_CORES)

    xT = nc.dram_tensor("xT", (D, S), f16, kind="ExternalInput").ap()
    wqT = nc.dram_tensor("wqT", (D, 256), f16, kind="ExternalInput").ap()
    wkT = nc.dram_tensor("wkT", (D, 256), f16, kind="ExternalInput").ap()
    wvT = nc.dram_tensor("wvT", (D, 256), f16, kind="ExternalInput").ap()
    woT = nc.dram_tensor("woT", (256, D), f16, kind="ExternalInput").ap()
    outT = nc.dram_tensor("outT", (D, S), f16, kind="ExternalOutput").ap()
    if dbg:
        d_qT = nc.dram_tensor("d_qT", (256, S), f16, kind="ExternalOutput").ap()
        d_kT = nc.dram_tensor("d_kT", (256, S), f16, kind="ExternalOutput").ap()
        d_v = nc.dram_tensor("d_v", (S, 258), f16, kind="ExternalOutput").ap()
        d_oT = nc.dram_tensor("d_oT", (256, S), f16, kind="ExternalOutput").ap()
        d_u = nc.dram_tensor("d_u", (S, 2, 258), f32, kind="ExternalOutput").ap()

    with tile.TileContext(nc) as tc:
        with (
            tc.tile_pool(name="const", bufs=1) as cpool,
            tc.tile_pool(name="persist", bufs=1) as pp,
        ):
            ident = cpool.tile([128, 128], f16, tag="ident")
            make_identity(nc, ident)
            # mask[p, f] = 1 if p <= f else 0 (keys on partitions, q on free)
            maskt = cpool.tile([128, 128], f16, tag="maskt")
            nc.gpsimd.memset(maskt, 1.0)
            nc.gpsimd.affine_select(
                out=maskt, in_=maskt, compare_op=OP.is_ge, fill=0.0,
                base=0, pattern=[[1, 128]], channel_multiplier=-1,
            )
            # constants for Newton-Raphson rsqrt (fast-inverse-sqrt seed)
            magic_c = cpool.tile([128, 8], u32, tag="magic_c")
            nc.gpsimd.memset(magic_c, 0x5F3759DF)
            one_u = cpool.tile([128, 8], u32, tag="one_u")
            nc.gpsimd.memset(one_u, 1)

            wo_sb = pp.tile([128, 2, D], f16, tag="wo_sb")
            qT_sb = pp.tile([128, 2, S], f16, tag="qT_sb")
            kT_sb = pp.tile([128, 2, S], f16, tag="kT_sb")
            # v with a ones column appended per head: [v_h0 | 1 | v_h1 | 1]
            v_sb = pp.tile([128, ST, 258], f16, tag="v_sb")
            nc.vector.memset(v_sb[:, :, 128:129], 1.0)
            nc.vector.memset(v_sb[:, :, 257:258], 1.0)
            oT_sb = pp.tile([128, 2, S], f16, tag="oT_sb")

            nc.gpsimd.dma_start(
                wo_sb[:, :, :],
                woT.rearrange("(kt p) n -> p kt n", p=128)[:, :, :])

            # ---------------- projections ----------------
            from contextlib import ExitStack
            pin_ctx = ExitStack()
            pin = pin_ctx.enter_context(tc.tile_pool(name="proj_in", bufs=1))
            with (
                tc.tile_pool(name="ps_qk", bufs=3, space="PSUM") as ps_qk,
                tc.tile_pool(name="ps_v", bufs=2, space="PSUM") as ps_v,
            ):
                xT_sb = pin.tile([128, KT, S], f16, tag="xT_sb")
                wq_sb = pin.tile([128, KT, 256], f16, tag="wq_sb")
                wk_sb = pin.tile([128, KT, 256], f16, tag="wk_sb")
                wv_sb = pin.tile([128, KT, 256], f16, tag="wv_sb")
                xT_r = xT.rearrange("(kt p) s -> p kt s", p=128)
                wq_r = wqT.rearrange("(kt p) m -> p kt m", p=128)
                wk_r = wkT.rearrange("(kt p) m -> p kt m", p=128)
                wv_r = wvT.rearrange("(kt p) m -> p kt m", p=128)
                nc.scalar.dma_start(wq_sb[:, :, :], wq_r[:, :, :])
                nc.scalar.dma_start(wk_sb[:, :, :], wk_r[:, :, :])
                nc.gpsimd.dma_start(wv_sb[:, :, :], wv_r[:, :, :])
                for kt2 in range(4):
                    nc.sync.dma_start(xT_sb[:, 2 * kt2:2 * kt2 + 2, :],
                                      xT_r[:, 2 * kt2:2 * kt2 + 2, :])

                def proj_qk(w_sb, dst_sb, mt, evac_engine):
                    for half in range(2):
                        ps = ps_qk.tile([128, 1024], f32, tag="qk")
                        for kt in range(KT):
                            for ns in range(2):
                                nc.tensor.matmul(
                                    ps[:, ns * 512:(ns + 1) * 512],
                                    lhsT=w_sb[:, kt, mt * 128:(mt + 1) * 128],
                                    rhs=xT_sb[:, kt,
                                              half * 1024 + ns * 512:
                                              half * 1024 + (ns + 1) * 512],
                                    start=(kt == 0), stop=(kt == KT - 1),
                                )
                        dst = dst_sb[:, mt, half * 1024:(half + 1) * 1024]
                        if evac_engine == "act":
                            nc.scalar.copy(dst, ps[:])
                        else:
                            nc.vector.tensor_copy(dst, ps[:])

                proj_qk(wq_sb, qT_sb, 0, "vector")
                proj_qk(wk_sb, kT_sb, 0, "vector")
                proj_qk(wq_sb, qT_sb, 1, "vector")
                proj_qk(wk_sb, kT_sb, 1, "vector")

                for st in range(ST):
                    ps = ps_v.tile([128, 256], f32, tag="v")
                    for kt in range(KT):
                        nc.tensor.matmul(
                            ps[:],
                            lhsT=xT_sb[:, kt, st * 128:(st + 1) * 128],
                            rhs=wv_sb[:, kt, :],
                            start=(kt == 0), stop=(kt == KT - 1),
                        )
                    nc.vector.tensor_copy(v_sb[:, st, 0:128], ps[:, 0:128])
                    nc.vector.tensor_copy(v_sb[:, st, 129:257], ps[:, 128:256])

            if dbg:
                d_qT_r = d_qT.rearrange("(mt p) s -> p mt s", p=128)
                d_kT_r = d_kT.rearrange("(mt p) s -> p mt s", p=128)
                d_v_r = d_v.rearrange("(st p) c -> p st c", p=128)
                for mt in range(2):
                    nc.sync.dma_start(d_qT_r[:, mt, :], qT_sb[:, mt, :])
                    nc.sync.dma_start(d_kT_r[:, mt, :], kT_sb[:, mt, :])
                for st in range(ST):
                    nc.sync.dma_start(d_v_r[:, st, :], v_sb[:, st, :])

            pin_ctx.close()

            # ---------------- attention + per-strip out projection ----------
            with (
                tc.tile_pool(name="e0p", bufs=20) as e0pool,
                tc.tile_pool(name="e1p", bufs=20) as e1pool,
                tc.tile_pool(name="ps_s", bufs=2, space="PSUM") as ps_s,
                tc.tile_pool(name="ps_u", bufs=2, space="PSUM") as ps_u,
                tc.tile_pool(name="ps_o", bufs=2, space="PSUM") as ps_o,
                tc.tile_pool(name="nrm", bufs=8) as nrm,
                tc.tile_pool(name="nrm_big", bufs=3) as nrm_big,
                tc.tile_pool(name="osb", bufs=3) as osb,
                tc.tile_pool(name="out_sb", bufs=2) as out_pool,
            ):
                epools = {0: e0pool, 1: e1pool}
                outT_r = outT.rearrange("(mt p) s -> p mt s", p=128)

                def pv_qtile(h, s, i, e0_tiles, e1_tiles, oq_s, ss_s):
                    qt = 4 * s + i
                    up = ps_u.tile([128, 258], f32, tag="u")
                    for kt in range(qt + 1):
                        c = i * 128
                        vh = v_sb[:, kt, 129 * h:129 * h + 129]
                        nc.tensor.matmul(
                            up[:, 0:129],
                            lhsT=e0_tiles[kt][:, c:c + 128],
                            rhs=vh,
                            start=(kt == 0), stop=(kt == qt),
                        )
                        nc.tensor.matmul(
                            up[:, 129:258],
                            lhsT=e1_tiles[kt][:, 512 + c:512 + c + 128],
                            rhs=vh,
                            start=False, stop=(kt == qt),
                            skip_group_check=True,
                        )
                    # normalized diff: oq = u0/s0 - lam*u1/s1 (per-partition)
                    inv0 = nrm.tile([128, 1], f32, tag="inv0")
                    nc.vector.reciprocal(inv0, up[:, 128:129])
                    inv1 = nrm.tile([128, 1], f32, tag="inv1")
                    nc.vector.reciprocal(inv1, up[:, 257:258])
                    t1 = nrm.tile([128, 128], f32, tag="t1")
                    nc.vector.tensor_scalar(t1, up[:, 129:257], inv1, lam,
                                            OP.mult, OP.mult)
                    oq = oq_s[:, i, :]
                    nc.vector.scalar_tensor_tensor(
                        oq, up[:, 0:128], inv0, t1, OP.mult, OP.subtract)
                    sq = nrm.tile([128, 128], f32, tag="sq")
                    nc.vector.scalar_tensor_tensor(
                        sq, oq, 1.0, oq, OP.bypass, OP.mult,
                        accum_out=ss_s[:, i:i + 1])
                    if dbg:
                        ub = nrm.tile([128, 258], f32, tag="ub")
                        nc.vector.tensor_copy(ub, up[:])
                        nc.sync.dma_start(
                            d_u.rearrange("(qt p) h c -> p qt h c",
                                          p=128)[:, qt, h, :], ub[:])

                def norm_tail(h, s, oq_s, ss_s):
                    """Newton rsqrt over the strip's 4 q-tiles, then scale,
                    transpose and evacuate o^T."""
                    ms = nrm.tile([128, 4], f32, tag="ms")
                    il2 = 1.0 / ((1.0 - LAMBDA_INIT) ** 2)
                    nc.vector.tensor_scalar(ms, ss_s, il2 / 128.0, EPS * il2,
                                            OP.mult, OP.add)
                    y0 = nrm.tile([128, 4], u32, tag="y0")
                    nc.vector.tensor_tensor(y0, ms.bitcast(u32),
                                            one_u[:, 0:4],
                                            OP.logical_shift_right)
                    nc.vector.tensor_tensor(y0, magic_c[:, 0:4], y0,
                                            OP.subtract)
                    yf = y0.bitcast(f32)
                    t2 = nrm.tile([128, 4], f32, tag="t2")
                    r_all = nrm.tile([128, 4], f32, tag="r_all")
                    nc.vector.tensor_mul(t2, yf, yf)
                    nc.vector.tensor_mul(t2, t2, ms)
                    nc.vector.tensor_scalar(t2, t2, -0.5, 1.5, OP.mult, OP.add)
                    nc.vector.tensor_mul(r_all, yf, t2)
                    nc.vector.tensor_mul(t2, r_all, r_all)
                    nc.vector.tensor_mul(t2, t2, ms)
                    nc.vector.tensor_scalar(t2, t2, -0.5, 1.5, OP.mult, OP.add)
                    nc.vector.tensor_mul(r_all, r_all, t2)
                    for i in range(4):
                        qt = 4 * s + i
                        on = osb.tile([128, 128], f16, tag="on")
                        nc.vector.tensor_scalar(on, oq_s[:, i, :],
                                                r_all[:, i:i + 1], None,
                                                OP.mult)
                        pt = ps_o.tile([128, 128], f16, tag="o")
                        nc.tensor.transpose(pt, on, ident)
                        nc.vector.tensor_copy(
                            oT_sb[:, h, qt * 128:(qt + 1) * 128], pt[:])

                def emit_outproj(s):
                    ot = out_pool.tile([128, 8, 512], f16, tag="ot")
                    for mt in range(8):
                        ps = ps_o.tile([128, 512], f32, tag="o")
                        for kt in range(2):
                            nc.tensor.matmul(
                                ps[:],
                                lhsT=wo_sb[:, kt, mt * 128:(mt + 1) * 128],
                                rhs=oT_sb[:, kt, s * 512:(s + 1) * 512],
                                start=(kt == 0), stop=(kt == 1),
                            )
                        nc.vector.tensor_copy(ot[:, mt, :], ps[:])
                    nc.sync.dma_start(outT_r[:, :, s * 512:(s + 1) * 512], ot[:])

                for si, s in enumerate([0, 3, 1, 2]):
                    for h in range(2):
                        e0_tiles = {}
                        e1_tiles = {}
                        oq_s = nrm_big.tile([128, 4, 128], f32, tag="oq_s")
                        ss_s = nrm_big.tile([128, 4], f32, tag="ss_s")
                        for kt in range(4 * (s + 1)):
                            col0 = max(0, (kt - 4 * s) * 128)
                            pa = ps_s.tile([128, 1024], f32, tag="sc")
                            nc.tensor.matmul(
                                pa[:, col0:512],
                                lhsT=kT_sb[0:64, h, kt * 128:(kt + 1) * 128],
                                rhs=qT_sb[0:64, h, s * 512 + col0:(s + 1) * 512],
                                start=True, stop=True, tile_position=(0, 0),
                            )
                            nc.tensor.matmul(
                                pa[:, 512 + col0:1024],
                                lhsT=kT_sb[64:128, h, kt * 128:(kt + 1) * 128],
                                rhs=qT_sb[64:128, h, s * 512 + col0:(s + 1) * 512],
                                start=True, stop=True, tile_position=(64, 0),
                                skip_group_check=True,
                            )
                            ee = epools[h].tile([128, 1024], f16, tag="e")
                            # exp both half-heads in one ACT op (strided AP
                            # skips the invalid leading columns of each half)
                            w_ = 512 - col0
                            nc.scalar.activation(
                                ee.rearrange("p (b c) -> p b c", b=2)[:, :, col0:512],
                                pa.rearrange("p (b c) -> p b c", b=2)[:, :, col0:512],
                                AF.Exp)
                            if kt >= 4 * s:
                                c = col0
                                nc.gpsimd.tensor_mul(ee[:, c:c + 128],
                                                     ee[:, c:c + 128], maskt)
                                nc.gpsimd.tensor_mul(ee[:, 512 + c:512 + c + 128],
                                                     ee[:, 512 + c:512 + c + 128],
                                                     maskt)
                            e0_tiles[kt] = ee
                            e1_tiles[kt] = ee
                            if kt >= 4 * s:
                                pv_qtile(h, s, kt - 4 * s, e0_tiles, e1_tiles,
                                         oq_s, ss_s)
                        norm_tail(h, s, oq_s, ss_s)

                    emit_outproj(s)

            if dbg:
                d_oT_r = d_oT.rearrange("(mt p) s -> p mt s", p=128)
                for mt in range(2):
                    nc.sync.dma_start(d_oT_r[:, mt, :], oT_sb[:, mt, :])

    nc.compile()
    return nc


def _prep_inputs(x, Wq, Wk, Wv, Wo):
    """Build the 8 per-core input maps (host-side shard + transpose)."""
    f16 = np.float16
    xT = [np.ascontiguousarray(x[b].T).astype(f16) for b in range(B)]
    scale = HD ** -0.5
    in_maps = []
    for d in range(N_CORES):
        b, p = divmod(d, 4)
        r0 = 256 * p
        in_maps.append({
            "xT": xT[b],
            "wqT": np.ascontiguousarray(Wq[r0:r0 + 256, :].T * scale).astype(f16),
            "wkT": np.ascontiguousarray(Wk[r0:r0 + 256, :].T).astype(f16),
            "wvT": np.ascontiguousarray(Wv[r0:r0 + 256, :].T).astype(f16),
            "woT": np.ascontiguousarray(Wo[:, r0:r0 + 256].T).astype(f16),
        })
    return in_maps


_CACHED = {}


def _get_program(lam: float):
    # the program depends on inputs only through lam
    key = round(float(lam), 9)
    if key not in _CACHED:
        _CACHED[key] = _build_program(float(lam))
    return _CACHED[key]


def kernel(x, Wq, Wk, Wv, Wo, lq1, lk1, lq2, lk2):
    from concourse.bass_utils import run_bass_kernel_spmd

    x = np.asarray(x, dtype=np.float32)
    Wq = np.asarray(Wq, dtype=np.float32)
    Wk = np.asarray(Wk, dtype=np.float32)
    Wv = np.asarray(Wv, dtype=np.float32)
    Wo = np.asarray(Wo, dtype=np.float32)
    lq1 = np.asarray(lq1, dtype=np.float32)
    lk1 = np.asarray(lk1, dtype=np.float32)
    lq2 = np.asarray(lq2, dtype=np.float32)
    lk2 = np.asarray(lk2, dtype=np.float32)

    lam1 = np.exp(np.sum(lq1 * lk1, dtype=np.float32))
    lam2 = np.exp(np.sum(lq2 * lk2, dtype=np.float32))
    lam = float(lam1 - lam2 + LAMBDA_INIT)

    nc = _get_program(lam)
    in_maps = _prep_inputs(x, Wq, Wk, Wv, Wo)
    res = run_bass_kernel_spmd(nc, in_maps, core_ids=list(range(N_CORES)))

    out = np.empty((B, S, D), dtype=np.float32)
    for b in range(B):
        acc = res.results[4 * b]["outT"].astype(np.float32)
        for p in range(1, 4):
            acc += res.results[4 * b + p]["outT"].astype(np.float32)
        out[b] = acc.T
    return out



# revision 8
# speedup vs baseline: 1.1620x; 1.1620x over previous
"""DiffAttention Trainium2 kernel (v2: fp8 hi/lo projections + pipelined schedule).

Full inputs in, full output out. Sharding: 8 cores = (batch b in {0,1}) x
(head-pair p in {0..3}); each core handles one batch element and 2 of the 8
heads (= 4 of the 16 q/k half-heads, 2 v heads, 256 of the 1024 o columns).
Out-projection is column-split: each core produces a full (S, D) partial of
o @ Wo.T restricted to its o columns; host sums the 4 partials per batch.

Projections run as fp8e4m3 DoubleRow matmuls with host-side error
compensation: x and each W shard are split hi/lo (hi = fp8(t), lo =
fp8(t - hi)) and the three significant products xh@Wh + xh@Wl + xl@Wh are
accumulated in PSUM (the dropped xl@Wl term is ~0.07% relative).  DoubleRow
contracts two 128-deep k-tiles per instruction at 0.5 cycles/row, so each
projection costs 6 rows/out-tile instead of fp16's 8.  Weights carry
power-of-2 pre-scales (q: 2^7*hd^-0.5, k/v: 2^4) to center fp8 exponents;
the combined 2^-11 is folded into the exp's scale argument and the RMSNorm
epsilon (the norm itself is scale-invariant), so no evacuation rescale is
needed anywhere.

Attention math per head h (half-heads e0=2h, e1=2h+1), per q row:
  u_i = exp(s_i) @ v   (unnormalized), sum_i = exp(s_i) @ 1  (fused: rhs=[v|1])
  o   = u0/sum0 - lam * u1/sum1
  o   = o * rsqrt(mean(o^2)+eps) * (1-lam_init);   out = o @ Wo.T
Scores are computed transposed (keys on partitions, q on free dim) so the
exp'd tiles feed the PV matmul directly as the stationary operand.  rsqrt is
Newton-Raphson on the DVE (fast-inverse-sqrt seed), batched per (strip, head).

Scheduling: a single flat unit stream, strip order [0,2,3,1] (small strip
last to shrink the drain tail).  Each strip's score+exp units are
interleaved with filler PE work (previous strip's PV chains / norm tails /
out-projection chunks, projection units, v-projection tiles) so the PE never
waits on the ACT exp pipeline and never idles long enough to drop out of its
high p-state.  Input DMA is chunked (x in 512-column blocks, weights
hi-before-lo) and ordered by first use so the first projection unit starts
~3.7us in.  PSUM: scores 2x[128,1024] + PV 2x[128,258] + a shared
[128,512] ring (projection evac / out-projection / f32 transposes, which
stay at 1 cycle/row because the identity operand is f16) = exactly 8 banks.
"""

import math

import numpy as np

B = 2
S = 2048
D = 1024
H = 8
HD = 64  # half-head dim
LAMBDA_INIT = 0.8 - 0.6 * math.exp(-0.3 * 6)
EPS = 1e-5

N_CORES = 8
KT = D // 128      # 8 contraction tiles for projections
ST = S // 128      # 16 sequence tiles
NSTRIP = S // 512  # 4 q strips

QSCALE = 2.0 ** 7   # folded into WqT (on top of hd^-0.5)
KSCALE = 2.0 ** 4   # folded into WkT
VSCALE = 2.0 ** 4   # folded into WvT
SSCALE = 1.0 / (QSCALE * KSCALE)   # exp() input scale
OSCALE2 = float(VSCALE * VSCALE)   # o is VSCALE-scaled; ss is VSCALE^2-scaled


def _build_program(lam: float):
    import concourse.bass as bass
    import concourse.tile as tile
    from concourse import bacc, mybir
    from concourse.masks import make_identity

    f8 = mybir.dt.float8e4
    f16 = mybir.dt.float16
    f32 = mybir.dt.float32
    u32 = mybir.dt.uint32
    AF = mybir.ActivationFunctionType
    OP = mybir.AluOpType
    DR = mybir.MatmulPerfMode.DoubleRow

    nc = bacc.Bacc("TRN2", target_bir_lowering=False, debug=False,
                   num_devices=N_CORES)

    xh_d = nc.dram_tensor("xh", (D, S), f8, kind="ExternalInput").ap()
    xl_d = nc.dram_tensor("xl", (D, S), f8, kind="ExternalInput").ap()
    # weights host-interleaved to [128, KT*256] for contiguous 2KB DMA runs
    w_d = {}
    for nm in ("wqh", "wql", "wkh", "wkl", "wvh", "wvl"):
        w_d[nm] = nc.dram_tensor(nm, (128, KT * 256), f8,
                                 kind="ExternalInput").ap()
    woT = nc.dram_tensor("woT", (256, D), f16, kind="ExternalInput").ap()
    outT = nc.dram_tensor("outT", (D, S), f16, kind="ExternalOutput").ap()

    with tile.TileContext(nc) as tc:
        with (
            tc.tile_pool(name="const", bufs=1) as cpool,
            tc.tile_pool(name="persist", bufs=1) as pp,
            tc.tile_pool(name="pin", bufs=1) as pin,
            tc.tile_pool(name="e0p", bufs=24) as e0pool,
            tc.tile_pool(name="e1p", bufs=24) as e1pool,
            tc.tile_pool(name="sc", bufs=2, space="PSUM") as sc_pool,
            tc.tile_pool(name="up", bufs=2, space="PSUM") as up_pool,
            tc.tile_pool(name="pj", bufs=2, space="PSUM") as pj_pool,
            tc.tile_pool(name="nrm", bufs=8) as nrm,
            tc.tile_pool(name="nrm_big", bufs=2) as nrm_big,
            tc.tile_pool(name="osb", bufs=3) as osb,
            tc.tile_pool(name="otp", bufs=2) as otp,
        ):
            epools = {0: e0pool, 1: e1pool}

            ident = cpool.tile([128, 128], f16, tag="ident")
            make_identity(nc, ident)
            # mask[p, f] = 1 if p <= f else 0 (keys on partitions, q on free)
            maskt = cpool.tile([128, 128], f16, tag="maskt")
            nc.gpsimd.memset(maskt, 1.0)
            nc.gpsimd.affine_select(
                out=maskt, in_=maskt, compare_op=OP.is_ge, fill=0.0,
                base=0, pattern=[[1, 128]], channel_multiplier=-1,
            )
            # constants for Newton-Raphson rsqrt (fast-inverse-sqrt seed)
            magic_c = cpool.tile([128, 8], u32, tag="magic_c")
            nc.gpsimd.memset(magic_c, 0x5F3759DF)
            one_u = cpool.tile([128, 8], u32, tag="one_u")
            nc.gpsimd.memset(one_u, 1)

            qT_sb = pp.tile([128, 2, S], f16, tag="qT_sb")
            kT_sb = pp.tile([128, 2, S], f16, tag="kT_sb")
            v_sb = pp.tile([128, ST, 258], f16, tag="v_sb")
            nc.vector.memset(v_sb[:, :, 128:129], 1.0)
            nc.vector.memset(v_sb[:, :, 257:258], 1.0)
            oT_sb = pp.tile([128, 2, S], f16, tag="oT_sb")
            wo_sb = pp.tile([128, 2, D], f16, tag="wo_sb")

            xh_sb = pin.tile([128, KT, S], f8, tag="xh_sb")
            xl_sb = pin.tile([128, KT, S], f8, tag="xl_sb")
            w_sb = {}
            for nm in ("wqh", "wql", "wkh", "wkl", "wvh", "wvl"):
                wt = pin.tile([128, KT, 256], f8, tag=nm + "_sb",
                              name=nm + "_sb")
                w_sb[nm] = wt

            # ---------------- input DMAs, ordered by first use -------------
            xh_r = xh_d.rearrange("(kt p) s -> p kt s", p=128)
            xl_r = xl_d.rearrange("(kt p) s -> p kt s", p=128)

            def dma_w(nm):
                nc.sync.dma_start(
                    w_sb[nm][:, :, :],
                    w_d[nm].rearrange("p (kt m) -> p kt m", m=256)[:, :, :])

            def dma_x(b):
                c0, c1 = 512 * b, 512 * (b + 1)
                nc.sync.dma_start(xh_sb[:, :, c0:c1], xh_r[:, :, c0:c1])
                nc.sync.dma_start(xl_sb[:, :, c0:c1], xl_r[:, :, c0:c1])

            dma_w("wkh")
            dma_x(0)
            dma_w("wkl")
            dma_w("wqh")
            dma_w("wql")
            dma_w("wvh")
            dma_w("wvl")
            dma_x(1)
            dma_x(2)
            dma_x(3)
            nc.gpsimd.dma_start(
                wo_sb[:, :, :],
                woT.rearrange("(kt p) n -> p kt n", p=128)[:, :, :])

            # ---------------- unit closures ----------------
            e_tiles = {}     # (s, h, kt) -> SBUF tile [128, 1024] f16
            oq_tiles = {}    # (s, h) -> [128, 4, 128] f32
            ss_tiles = {}    # (s, h) -> [128, 4] f32
            ot_tiles = {}    # s -> [128, 8, 512] f16
            outT_r = outT.rearrange("(mt p) s -> p mt s", p=128)

            def proj_qk(wh, wl, dst_sb, h, b):
                """qT/kT columns [b*512,(b+1)*512) for local head h."""
                ps = pj_pool.tile([128, 512], f32, tag="pj")
                c0, c1 = 512 * b, 512 * (b + 1)
                m0, m1 = 128 * h, 128 * (h + 1)
                chains = ((wh, xh_sb), (wh, xl_sb), (wl, xh_sb))
                for ci, (w, x) in enumerate(chains):
                    for pr in range(KT // 2):
                        nc.tensor.matmul(
                            ps[:, :],
                            lhsT=w_sb[w][:, 2 * pr:2 * pr + 2, m0:m1],
                            rhs=x[:, 2 * pr:2 * pr + 2, c0:c1],
                            start=(ci == 0 and pr == 0),
                            stop=(ci == 2 and pr == KT // 2 - 1),
                            perf_mode=DR,
                        )
                nc.scalar.copy(dst_sb[:, h, c0:c1], ps[:, :])

            def proj_v(st):
                """v rows [st*128,(st+1)*128) for both local heads."""
                ps = pj_pool.tile([128, 512], f32, tag="pj")
                r0, r1 = 128 * st, 128 * (st + 1)
                chains = (("wvh", xh_sb), ("wvh", xl_sb), ("wvl", xh_sb))
                for ci, (w, x) in enumerate(chains):
                    for pr in range(KT // 2):
                        nc.tensor.matmul(
                            ps[:, 0:256],
                            lhsT=x[:, 2 * pr:2 * pr + 2, r0:r1],
                            rhs=w_sb[w][:, 2 * pr:2 * pr + 2, :],
                            start=(ci == 0 and pr == 0),
                            stop=(ci == 2 and pr == KT // 2 - 1),
                            perf_mode=DR,
                        )
                nc.vector.tensor_copy(v_sb[:, st, 0:128], ps[:, 0:128])
                nc.vector.tensor_copy(v_sb[:, st, 129:257], ps[:, 128:256])

            def scores(s, h, kt):
                col0 = max(0, (kt - 4 * s) * 128)
                pa = sc_pool.tile([128, 1024], f32, tag="sc")
                nc.tensor.matmul(
                    pa[:, col0:512],
                    lhsT=kT_sb[0:64, h, kt * 128:(kt + 1) * 128],
                    rhs=qT_sb[0:64, h, s * 512 + col0:(s + 1) * 512],
                    start=True, stop=True, tile_position=(0, 0),
                )
                nc.tensor.matmul(
                    pa[:, 512 + col0:1024],
                    lhsT=kT_sb[64:128, h, kt * 128:(kt + 1) * 128],
                    rhs=qT_sb[64:128, h, s * 512 + col0:(s + 1) * 512],
                    start=True, stop=True, tile_position=(64, 0),
                    skip_group_check=True,
                )
                ee = epools[h].tile([128, 1024], f16, tag="e")
                nc.scalar.activation(
                    ee.rearrange("p (b c) -> p b c", b=2)[:, :, col0:512],
                    pa.rearrange("p (b c) -> p b c", b=2)[:, :, col0:512],
                    AF.Exp, scale=SSCALE)
                if kt >= 4 * s:
                    c = col0
                    nc.gpsimd.tensor_mul(ee[:, c:c + 128],
                                         ee[:, c:c + 128], maskt)
                    nc.gpsimd.tensor_mul(ee[:, 512 + c:512 + c + 128],
                                         ee[:, 512 + c:512 + c + 128],
                                         maskt)
                e_tiles[(s, h, kt)] = ee

            def pv(s, h, i):
                """PV chain + per-qtile normalized diff for qtile i of strip s."""
                qt = 4 * s + i
                if (s, h) not in oq_tiles:
                    oq_tiles[(s, h)] = nrm_big.tile(
                        [128, 4, 128], f32, tag="oq_s", name=f"oq_{s}_{h}")
                    ss_tiles[(s, h)] = nrm_big.tile(
                        [128, 4], f32, tag="ss_s", name=f"ss_{s}_{h}")
                oq_s = oq_tiles[(s, h)]
                ss_s = ss_tiles[(s, h)]
                up = up_pool.tile([128, 258], f32, tag="up")
                c = i * 128
                for kt in range(qt + 1):
                    ee = e_tiles[(s, h, kt)]
                    vh = v_sb[:, kt, 129 * h:129 * h + 129]
                    nc.tensor.matmul(
                        up[:, 0:129],
                        lhsT=ee[:, c:c + 128],
                        rhs=vh,
                        start=(kt == 0), stop=(kt == qt),
                    )
                    nc.tensor.matmul(
                        up[:, 129:258],
                        lhsT=ee[:, 512 + c:512 + c + 128],
                        rhs=vh,
                        start=False, stop=(kt == qt),
                        skip_group_check=True,
                    )
                inv0 = nrm.tile([128, 1], f32, tag="inv0")
                nc.vector.reciprocal(inv0, up[:, 128:129])
                inv1 = nrm.tile([128, 1], f32, tag="inv1")
                nc.vector.reciprocal(inv1, up[:, 257:258])
                t1 = nrm.tile([128, 128], f32, tag="t1")
                nc.vector.tensor_scalar(t1, up[:, 129:257], inv1, lam,
                                        OP.mult, OP.mult)
                oq = oq_s[:, i, :]
                nc.vector.scalar_tensor_tensor(
                    oq, up[:, 0:128], inv0, t1, OP.mult, OP.subtract)
                sq = nrm.tile([128, 128], f32, tag="sq")
                nc.vector.scalar_tensor_tensor(
                    sq, oq, 1.0, oq, OP.bypass, OP.mult,
                    accum_out=ss_s[:, i:i + 1])

            def norm_tail(s, h):
                """Newton rsqrt over the strip's 4 q-tiles, scale, transpose."""
                oq_s = oq_tiles[(s, h)]
                ss_s = ss_tiles[(s, h)]
                ms = nrm.tile([128, 4], f32, tag="ms")
                il2 = 1.0 / ((1.0 - LAMBDA_INIT) ** 2)
                nc.vector.tensor_scalar(ms, ss_s, il2 / 128.0,
                                        EPS * il2 * OSCALE2,
                                        OP.mult, OP.add)
                y0 = nrm.tile([128, 4], u32, tag="y0")
                nc.vector.tensor_tensor(y0, ms.bitcast(u32), one_u[:, 0:4],
                                        OP.logical_shift_right)
                nc.vector.tensor_tensor(y0, magic_c[:, 0:4], y0, OP.subtract)
                yf = y0.bitcast(f32)
                t2 = nrm.tile([128, 4], f32, tag="t2")
                r_all = nrm.tile([128, 4], f32, tag="r_all")
                nc.vector.tensor_mul(t2, yf, yf)
                nc.vector.tensor_mul(t2, t2, ms)
                nc.vector.tensor_scalar(t2, t2, -0.5, 1.5, OP.mult, OP.add)
                nc.vector.tensor_mul(r_all, yf, t2)
                nc.vector.tensor_mul(t2, r_all, r_all)
                nc.vector.tensor_mul(t2, t2, ms)
                nc.vector.tensor_scalar(t2, t2, -0.5, 1.5, OP.mult, OP.add)
                nc.vector.tensor_mul(r_all, r_all, t2)
                for i in range(4):
                    qt = 4 * s + i
                    on = osb.tile([128, 128], f16, tag="on")
                    nc.vector.tensor_scalar(on, oq_s[:, i, :],
                                            r_all[:, i:i + 1], None, OP.mult)
                    pt = pj_pool.tile([128, 512], f32, tag="pj")
                    ptv = pt.bitcast(f16)
                    nc.tensor.transpose(ptv[:, 0:128], on, ident)
                    nc.vector.tensor_copy(
                        oT_sb[:, h, qt * 128:(qt + 1) * 128], ptv[:, 0:128])

            def outproj(s, mt):
                if s not in ot_tiles:
                    ot_tiles[s] = otp.tile([128, 8, 512], f16, tag="ot",
                                           name=f"ot_{s}")
                ot = ot_tiles[s]
                ps = pj_pool.tile([128, 512], f32, tag="pj")
                for kt2 in range(2):
                    nc.tensor.matmul(
                        ps[:],
                        lhsT=wo_sb[:, kt2, mt * 128:(mt + 1) * 128],
                        rhs=oT_sb[:, kt2, s * 512:(s + 1) * 512],
                        start=(kt2 == 0), stop=(kt2 == 1),
                    )
                nc.vector.tensor_copy(ot[:, mt, :], ps[:])
                if mt == 7:
                    nc.sync.dma_start(
                        outT_r[:, :, s * 512:(s + 1) * 512], ot[:])

            # ---------------- flat schedule ----------------
            U = []  # list of thunks

            def k_u(h, b):
                return lambda: proj_qk("wkh", "wkl", kT_sb, h, b)

            def q_u(h, b):
                return lambda: proj_qk("wqh", "wql", qT_sb, h, b)

            def v_u(st):
                return lambda: proj_v(st)

            def sc_u(s, h, kt):
                return lambda: scores(s, h, kt)

            def pv_u(s, h, i):
                return lambda: pv(s, h, i)

            def nt_u(s, h):
                return lambda: norm_tail(s, h)

            def op_u(s, mt):
                return lambda: outproj(s, mt)

            def a_phase(s, fillers):
                fi = iter(fillers)
                for kt in range(4 * s + 4):
                    U.append(sc_u(s, 0, kt))
                    U.append(sc_u(s, 1, kt))
                    for _ in range(2):
                        f = next(fi, None)
                        if f is not None:
                            U.append(f)
                rest = list(fi)
                U.extend(rest)

            U += [k_u(0, 0), k_u(1, 0), q_u(0, 0), q_u(1, 0)]

            a_phase(0, [v_u(0), v_u(1), v_u(2), v_u(3),
                        k_u(0, 1), k_u(1, 1), q_u(0, 2), q_u(1, 2)])

            a_phase(2, [pv_u(0, 0, 0), pv_u(0, 1, 0),
                        pv_u(0, 0, 1), pv_u(0, 1, 1),
                        k_u(0, 2), k_u(1, 2),
                        pv_u(0, 0, 2), pv_u(0, 1, 2),
                        pv_u(0, 0, 3), pv_u(0, 1, 3),
                        nt_u(0, 0), nt_u(0, 1),
                        q_u(0, 3), q_u(1, 3),
                        v_u(4), v_u(5), v_u(6), v_u(7), v_u(8), v_u(9),
                        v_u(10), v_u(11),
                        op_u(0, 0), op_u(0, 1)])

            a_phase(3, [k_u(0, 3), k_u(1, 3),
                        op_u(0, 2), op_u(0, 3),
                        pv_u(2, 0, 0), pv_u(2, 1, 0),
                        pv_u(2, 0, 1), pv_u(2, 1, 1),
                        op_u(0, 4), op_u(0, 5),
                        pv_u(2, 0, 2), pv_u(2, 1, 2),
                        pv_u(2, 0, 3), pv_u(2, 1, 3),
                        nt_u(2, 0), nt_u(2, 1),
                        op_u(0, 6), op_u(0, 7),
                        op_u(2, 0), op_u(2, 1), op_u(2, 2), op_u(2, 3),
                        op_u(2, 4), op_u(2, 5), op_u(2, 6), op_u(2, 7),
                        q_u(0, 1), q_u(1, 1),
                        v_u(12), v_u(13), v_u(14), v_u(15)])

            a_phase(1, [pv_u(3, 0, 0), pv_u(3, 1, 0),
                        pv_u(3, 0, 1), pv_u(3, 1, 1),
                        pv_u(3, 0, 2), pv_u(3, 1, 2),
                        pv_u(3, 0, 3), pv_u(3, 1, 3),
                        nt_u(3, 0), nt_u(3, 1),
                        op_u(3, 0), op_u(3, 1), op_u(3, 2), op_u(3, 3),
                        op_u(3, 4), op_u(3, 5), op_u(3, 6), op_u(3, 7)])

            U += [pv_u(1, 0, 0), pv_u(1, 1, 0),
                  pv_u(1, 0, 1), pv_u(1, 1, 1),
                  pv_u(1, 0, 2), pv_u(1, 1, 2),
                  pv_u(1, 0, 3), pv_u(1, 1, 3),
                  nt_u(1, 0), nt_u(1, 1),
                  op_u(1, 0), op_u(1, 1), op_u(1, 2), op_u(1, 3),
                  op_u(1, 4), op_u(1, 5), op_u(1, 6), op_u(1, 7)]

            for u in U:
                u()

    nc.compile()
    return nc


def _split_fp8(a):
    import ml_dtypes
    f8 = ml_dtypes.float8_e4m3
    hi = a.astype(f8)
    lo = (a - hi.astype(np.float32)).astype(f8)
    return hi, lo


def _interleave_w(wT):
    """[D, 256] -> [128, KT*256] with arr[p, kt*256+m] = wT[kt*128+p, m]."""
    return np.ascontiguousarray(
        wT.reshape(KT, 128, 256).transpose(1, 0, 2).reshape(128, KT * 256))


def _prep_inputs(x, Wq, Wk, Wv, Wo):
    """Build the 8 per-core input maps (host-side shard/split/transpose)."""
    f16 = np.float16
    scale = HD ** -0.5
    xs = []
    for b in range(B):
        xT = np.ascontiguousarray(x[b].T).astype(np.float32)
        xs.append(_split_fp8(xT))
    in_maps = []
    for d in range(N_CORES):
        b, p = divmod(d, 4)
        r0 = 256 * p
        xh, xl = xs[b]
        wq = np.ascontiguousarray(Wq[r0:r0 + 256, :].T) * (scale * QSCALE)
        wk = np.ascontiguousarray(Wk[r0:r0 + 256, :].T) * KSCALE
        wv = np.ascontiguousarray(Wv[r0:r0 + 256, :].T) * VSCALE
        wqh, wql = _split_fp8(wq.astype(np.float32))
        wkh, wkl = _split_fp8(wk.astype(np.float32))
        wvh, wvl = _split_fp8(wv.astype(np.float32))
        in_maps.append({
            "xh": xh, "xl": xl,
            "wqh": _interleave_w(wqh), "wql": _interleave_w(wql),
            "wkh": _interleave_w(wkh), "wkl": _interleave_w(wkl),
            "wvh": _interleave_w(wvh), "wvl": _interleave_w(wvl),
            "woT": np.ascontiguousarray(Wo[:, r0:r0 + 256].T).astype(f16),
        })
    return in_maps


_CACHED = {}


def _get_program(lam: float):
    # the program depends on inputs only through lam
    key = round(float(lam), 9)
    if key not in _CACHED:
        _CACHED[key] = _build_program(float(lam))
    return _CACHED[key]


def kernel(x, Wq, Wk, Wv, Wo, lq1, lk1, lq2, lk2):
    from concourse.bass_utils import run_bass_kernel_spmd

    x = np.asarray(x, dtype=np.float32)
    Wq = np.asarray(Wq, dtype=np.float32)
    Wk = np.asarray(Wk, dtype=np.float32)
    Wv = np.asarray(Wv, dtype=np.float32)
    Wo = np.asarray(Wo, dtype=np.float32)
    lq1 = np.asarray(lq1, dtype=np.float32)
    lk1 = np.asarray(lk1, dtype=np.float32)
    lq2 = np.asarray(lq2, dtype=np.float32)
    lk2 = np.asarray(lk2, dtype=np.float32)

    lam1 = np.exp(np.sum(lq1 * lk1, dtype=np.float32))
    lam2 = np.exp(np.sum(lq2 * lk2, dtype=np.float32))
    lam = float(lam1 - lam2 + LAMBDA_INIT)

    nc = _get_program(lam)
    in_maps = _prep_inputs(x, Wq, Wk, Wv, Wo)
    res = run_bass_kernel_spmd(nc, in_maps, core_ids=list(range(N_CORES)))

    out = np.empty((B, S, D), dtype=np.float32)
    for b in range(B):
        acc = res.results[4 * b]["outT"].astype(np.float32)
        for p in range(1, 4):
            acc += res.results[4 * b + p]["outT"].astype(np.float32)
        out[b] = acc.T
    return out


# revision 35
# speedup vs baseline: 1.1768x; 1.0127x over previous
"""DiffAttention Trainium2 kernel (v2: fp8 hi/lo projections + pipelined schedule).

Full inputs in, full output out. Sharding: 8 cores = (batch b in {0,1}) x
(head-pair p in {0..3}); each core handles one batch element and 2 of the 8
heads (= 4 of the 16 q/k half-heads, 2 v heads, 256 of the 1024 o columns).
Out-projection is column-split: each core produces a full (S, D) partial of
o @ Wo.T restricted to its o columns; host sums the 4 partials per batch.

Projections run as fp8e4m3 DoubleRow matmuls with host-side error
compensation: x and each W shard are split hi/lo (hi = fp8(t), lo =
fp8(t - hi)) and the three significant products xh@Wh + xh@Wl + xl@Wh are
accumulated in PSUM (the dropped xl@Wl term is ~0.07% relative).  DoubleRow
contracts two 128-deep k-tiles per instruction at 0.5 cycles/row, so each
projection costs 6 rows/out-tile instead of fp16's 8.  Weights carry
power-of-2 pre-scales (q: 2^7*hd^-0.5, k/v: 2^4) to center fp8 exponents;
the combined 2^-11 is folded into the exp's scale argument and the RMSNorm
epsilon (the norm itself is scale-invariant), so no evacuation rescale is
needed anywhere.

Attention math per head h (half-heads e0=2h, e1=2h+1), per q row:
  u_i = exp(s_i) @ v   (unnormalized), sum_i = exp(s_i) @ 1  (fused: rhs=[v|1])
  o   = u0/sum0 - lam * u1/sum1
  o   = o * rsqrt(mean(o^2)+eps) * (1-lam_init);   out = o @ Wo.T
Scores are computed transposed (keys on partitions, q on free dim) so the
exp'd tiles feed the PV matmul directly as the stationary operand.  rsqrt is
Newton-Raphson on the DVE (fast-inverse-sqrt seed), batched per (strip, head).

Scheduling: a single flat unit stream, strip order [0,2,3,1] (small strip
last to shrink the drain tail).  Each strip's score+exp units are
interleaved with filler PE work (previous strip's PV chains / norm tails /
out-projection chunks, projection units, v-projection tiles) so the PE never
waits on the ACT exp pipeline and never idles long enough to drop out of its
high p-state.  Input DMA is chunked (x in 512-column blocks, weights
hi-before-lo) and ordered by first use so the first projection unit starts
~3.7us in.  PSUM: scores 2x[128,1024] + PV 2x[128,258] + a shared
[128,512] ring (projection evac / out-projection / f32 transposes, which
stay at 1 cycle/row because the identity operand is f16) = exactly 8 banks.
"""

import math

import numpy as np

B = 2
S = 2048
D = 1024
H = 8
HD = 64  # half-head dim
LAMBDA_INIT = 0.8 - 0.6 * math.exp(-0.3 * 6)
EPS = 1e-5

N_CORES = 8
KT = D // 128      # 8 contraction tiles for projections
ST = S // 128      # 16 sequence tiles
NSTRIP = S // 512  # 4 q strips

QSCALE = 2.0 ** 7   # folded into WqT (on top of hd^-0.5)
KSCALE = 2.0 ** 4   # folded into WkT
VSCALE = 2.0 ** 4   # folded into WvT
SSCALE = 1.0 / (QSCALE * KSCALE)   # exp() input scale
OSCALE2 = float(VSCALE * VSCALE)   # o is VSCALE-scaled; ss is VSCALE^2-scaled


def _build_program(lam: float):
    import concourse.bass as bass
    import concourse.tile as tile
    from concourse import bacc, mybir
    from concourse.masks import make_identity

    f8 = mybir.dt.float8e4
    f16 = mybir.dt.float16
    f32 = mybir.dt.float32
    u32 = mybir.dt.uint32
    AF = mybir.ActivationFunctionType
    OP = mybir.AluOpType
    DR = mybir.MatmulPerfMode.DoubleRow

    nc = bacc.Bacc("TRN2", target_bir_lowering=False, debug=False,
                   num_devices=N_CORES)

    xh_d = nc.dram_tensor("xh", (D, S), f8, kind="ExternalInput").ap()
    xl_d = nc.dram_tensor("xl", (D, S), f8, kind="ExternalInput").ap()
    # weights host-interleaved to [128, KT*256] for contiguous 2KB DMA runs
    w_d = {}
    for nm in ("wqh", "wql", "wkh", "wkl", "wvh", "wvl"):
        w_d[nm] = nc.dram_tensor(nm, (128, KT * 256), f8,
                                 kind="ExternalInput").ap()
    woT = nc.dram_tensor("woT", (256, D), f16, kind="ExternalInput").ap()
    outT = nc.dram_tensor("outT", (D, S), f16, kind="ExternalOutput").ap()

    with tile.TileContext(nc) as tc:
        with (
            tc.tile_pool(name="const", bufs=1) as cpool,
            tc.tile_pool(name="persist", bufs=1) as pp,
            tc.tile_pool(name="pin", bufs=1) as pin,
            tc.tile_pool(name="e0p", bufs=24) as e0pool,
            tc.tile_pool(name="e1p", bufs=24) as e1pool,
            tc.tile_pool(name="sc", bufs=2, space="PSUM") as sc_pool,
            tc.tile_pool(name="up", bufs=2, space="PSUM") as up_pool,
            tc.tile_pool(name="pj", bufs=2, space="PSUM") as pj_pool,
            tc.tile_pool(name="nrm", bufs=8) as nrm,
            tc.tile_pool(name="nrm_big", bufs=2) as nrm_big,
            tc.tile_pool(name="osb", bufs=3) as osb,
            tc.tile_pool(name="otp", bufs=2) as otp,
        ):
            epools = {0: e0pool, 1: e1pool}

            ident = cpool.tile([128, 128], f16, tag="ident")
            make_identity(nc, ident)
            # mask[p, f] = 1 if p <= f else 0 (keys on partitions, q on free)
            maskt = cpool.tile([128, 128], f16, tag="maskt")
            nc.gpsimd.memset(maskt, 1.0)
            nc.gpsimd.affine_select(
                out=maskt, in_=maskt, compare_op=OP.is_ge, fill=0.0,
                base=0, pattern=[[1, 128]], channel_multiplier=-1,
            )
            # constants for Newton-Raphson rsqrt (fast-inverse-sqrt seed)
            magic_c = cpool.tile([128, 8], u32, tag="magic_c")
            nc.gpsimd.memset(magic_c, 0x5F3759DF)
            one_u = cpool.tile([128, 8], u32, tag="one_u")
            nc.gpsimd.memset(one_u, 1)

            qT_sb = pp.tile([128, 2, S], f16, tag="qT_sb")
            kT_sb = pp.tile([128, 2, S], f16, tag="kT_sb")
            v_sb = pp.tile([128, ST, 258], f16, tag="v_sb")
            nc.vector.memset(v_sb[:, :, 128:129], 1.0)
            nc.vector.memset(v_sb[:, :, 257:258], 1.0)
            oT_sb = pp.tile([128, 2, S], f16, tag="oT_sb")
            wo_sb = pp.tile([128, 2, D], f16, tag="wo_sb")

            xh_sb = pin.tile([128, KT, S], f8, tag="xh_sb")
            xl_sb = pin.tile([128, KT, S], f8, tag="xl_sb")
            w_sb = {}
            for nm in ("wqh", "wql", "wkh", "wkl", "wvh", "wvl"):
                wt = pin.tile([128, KT, 256], f8, tag=nm + "_sb",
                              name=nm + "_sb")
                w_sb[nm] = wt

            # ---------------- input DMAs, ordered by first use -------------
            xh_r = xh_d.rearrange("(kt p) s -> p kt s", p=128)
            xl_r = xl_d.rearrange("(kt p) s -> p kt s", p=128)

            def dma_w(nm):
                nc.sync.dma_start(
                    w_sb[nm][:, :, :],
                    w_d[nm].rearrange("p (kt m) -> p kt m", m=256)[:, :, :])

            def dma_x(b):
                c0, c1 = 512 * b, 512 * (b + 1)
                nc.sync.dma_start(xh_sb[:, :, c0:c1], xh_r[:, :, c0:c1])
                nc.sync.dma_start(xl_sb[:, :, c0:c1], xl_r[:, :, c0:c1])

            def dma_x_half(b, half):
                c0 = 512 * b + 256 * half
                c1 = c0 + 256
                nc.sync.dma_start(xh_sb[:, :, c0:c1], xh_r[:, :, c0:c1])
                nc.sync.dma_start(xl_sb[:, :, c0:c1], xl_r[:, :, c0:c1])

            dma_w("wkh")
            dma_x_half(0, 0)
            dma_w("wkl")
            dma_x_half(0, 1)
            dma_w("wqh")
            dma_w("wql")
            dma_w("wvh")
            dma_w("wvl")
            dma_x(1)
            dma_x(2)
            nc.gpsimd.dma_start(
                wo_sb[:, :, :],
                woT.rearrange("(kt p) n -> p kt n", p=128)[:, :, :])
            dma_x(3)

            # ---------------- unit closures ----------------
            e_tiles = {}     # (s, h, kt) -> SBUF tile [128, 1024] f16
            oq_tiles = {}    # (s, h) -> [128, 4, 128] f32
            ss_tiles = {}    # (s, h) -> [128, 4] f32
            outT_r = outT.rearrange("(mt p) s -> p mt s", p=128)

            def proj_qk(wh, wl, dst_sb, h, b, half=None):
                """qT/kT columns [b*512,(b+1)*512) for local head h."""
                ps = pj_pool.tile([128, 512], f32, tag="pj")
                if half is None:
                    c0, c1, w0 = 512 * b, 512 * (b + 1), 512
                else:
                    c0 = 512 * b + 256 * half
                    c1, w0 = c0 + 256, 256
                m0, m1 = 128 * h, 128 * (h + 1)
                chains = ((wh, xh_sb), (wh, xl_sb), (wl, xh_sb))
                for ci, (w, x) in enumerate(chains):
                    for pr in range(KT // 2):
                        nc.tensor.matmul(
                            ps[:, 0:w0],
                            lhsT=w_sb[w][:, 2 * pr:2 * pr + 2, m0:m1],
                            rhs=x[:, 2 * pr:2 * pr + 2, c0:c1],
                            start=(ci == 0 and pr == 0),
                            stop=(ci == 2 and pr == KT // 2 - 1),
                            perf_mode=DR,
                        )
                nc.vector.tensor_copy(dst_sb[:, h, c0:c1], ps[:, 0:w0])

            def proj_v(st):
                """v rows [st*128,(st+1)*128) for both local heads."""
                ps = pj_pool.tile([128, 512], f32, tag="pj")
                r0, r1 = 128 * st, 128 * (st + 1)
                chains = (("wvh", xh_sb), ("wvh", xl_sb), ("wvl", xh_sb))
                for ci, (w, x) in enumerate(chains):
                    for pr in range(KT // 2):
                        nc.tensor.matmul(
                            ps[:, 0:256],
                            lhsT=x[:, 2 * pr:2 * pr + 2, r0:r1],
                            rhs=w_sb[w][:, 2 * pr:2 * pr + 2, :],
                            start=(ci == 0 and pr == 0),
                            stop=(ci == 2 and pr == KT // 2 - 1),
                            perf_mode=DR,
                        )
                nc.vector.tensor_copy(v_sb[:, st, 0:128], ps[:, 0:128])
                nc.vector.tensor_copy(v_sb[:, st, 129:257], ps[:, 128:256])

            def scores(s, h, kt):
                col0 = max(0, (kt - 4 * s) * 128)
                pa = sc_pool.tile([128, 1024], f32, tag="sc")
                nc.tensor.matmul(
                    pa[:, col0:512],
                    lhsT=kT_sb[0:64, h, kt * 128:(kt + 1) * 128],
                    rhs=qT_sb[0:64, h, s * 512 + col0:(s + 1) * 512],
                    start=True, stop=True, tile_position=(0, 0),
                )
                nc.tensor.matmul(
                    pa[:, 512 + col0:1024],
                    lhsT=kT_sb[64:128, h, kt * 128:(kt + 1) * 128],
                    rhs=qT_sb[64:128, h, s * 512 + col0:(s + 1) * 512],
                    start=True, stop=True, tile_position=(64, 0),
                    skip_group_check=True,
                )
                ee = epools[h].tile([128, 1024], f16, tag="e")
                nc.scalar.activation(
                    ee.rearrange("p (b c) -> p b c", b=2)[:, :, col0:512],
                    pa.rearrange("p (b c) -> p b c", b=2)[:, :, col0:512],
                    AF.Exp, scale=SSCALE)
                if kt >= 4 * s:
                    c = col0
                    nc.gpsimd.tensor_mul(ee[:, c:c + 128],
                                         ee[:, c:c + 128], maskt)
                    nc.gpsimd.tensor_mul(ee[:, 512 + c:512 + c + 128],
                                         ee[:, 512 + c:512 + c + 128],
                                         maskt)
                e_tiles[(s, h, kt)] = ee

            def pv(s, h, i):
                """PV chain + per-qtile normalized diff for qtile i of strip s."""
                qt = 4 * s + i
                if (s, h) not in oq_tiles:
                    oq_tiles[(s, h)] = nrm_big.tile(
                        [128, 4, 128], f16, tag="oq_s", name=f"oq_{s}_{h}")
                    ss_tiles[(s, h)] = nrm_big.tile(
                        [128, 4], f32, tag="ss_s", name=f"ss_{s}_{h}")
                oq_s = oq_tiles[(s, h)]
                ss_s = ss_tiles[(s, h)]
                up = up_pool.tile([128, 258], f32, tag="up")
                c = i * 128
                for kt in range(qt + 1):
                    ee = e_tiles[(s, h, kt)]
                    vh = v_sb[:, kt, 129 * h:129 * h + 129]
                    nc.tensor.matmul(
                        up[:, 0:129],
                        lhsT=ee[:, c:c + 128],
                        rhs=vh,
                        start=(kt == 0), stop=(kt == qt),
                    )
                    nc.tensor.matmul(
                        up[:, 129:258],
                        lhsT=ee[:, 512 + c:512 + c + 128],
                        rhs=vh,
                        start=False, stop=(kt == qt),
                        skip_group_check=True,
                    )
                inv0 = nrm.tile([128, 1], f32, tag="inv0")
                nc.vector.reciprocal(inv0, up[:, 128:129])
                inv1 = nrm.tile([128, 1], f32, tag="inv1")
                nc.vector.reciprocal(inv1, up[:, 257:258])
                t1 = nrm.tile([128, 128], f32, tag="t1")
                nc.vector.tensor_scalar(t1, up[:, 129:257], inv1, lam,
                                        OP.mult, OP.mult)
                oq = oq_s[:, i, :]
                nc.vector.scalar_tensor_tensor(
                    oq, up[:, 0:128], inv0, t1, OP.mult, OP.subtract)
                sq = nrm.tile([128, 128], f16, tag="sq")
                nc.vector.scalar_tensor_tensor(
                    sq, oq, 1.0, oq, OP.bypass, OP.mult,
                    accum_out=ss_s[:, i:i + 1])

            def norm_tail(s, h):
                """Newton rsqrt over the strip's 4 q-tiles, scale, transpose."""
                oq_s = oq_tiles[(s, h)]
                ss_s = ss_tiles[(s, h)]
                ms = nrm.tile([128, 4], f32, tag="ms")
                il2 = 1.0 / ((1.0 - LAMBDA_INIT) ** 2)
                nc.vector.tensor_scalar(ms, ss_s, il2 / 128.0,
                                        EPS * il2 * OSCALE2,
                                        OP.mult, OP.add)
                y0 = nrm.tile([128, 4], u32, tag="y0")
                nc.vector.tensor_tensor(y0, ms.bitcast(u32), one_u[:, 0:4],
                                        OP.logical_shift_right)
                nc.vector.tensor_tensor(y0, magic_c[:, 0:4], y0, OP.subtract)
                yf = y0.bitcast(f32)
                t2 = nrm.tile([128, 4], f32, tag="t2")
                r_all = nrm.tile([128, 4], f32, tag="r_all")
                nc.vector.tensor_mul(t2, yf, yf)
                nc.vector.tensor_mul(t2, t2, ms)
                nc.vector.tensor_scalar(t2, t2, -0.5, 1.5, OP.mult, OP.add)
                nc.vector.tensor_mul(r_all, yf, t2)
                nc.vector.tensor_mul(t2, r_all, r_all)
                nc.vector.tensor_mul(t2, t2, ms)
                nc.vector.tensor_scalar(t2, t2, -0.5, 1.5, OP.mult, OP.add)
                nc.vector.tensor_mul(r_all, r_all, t2)
                for i in range(4):
                    qt = 4 * s + i
                    on = osb.tile([128, 128], f16, tag="on")
                    if s == 1:
                        # tail strip: ACT is exp-idle by now; keep DVE clear
                        nc.scalar.activation(on, oq_s[:, i, :], AF.Copy,
                                             scale=r_all[:, i:i + 1])
                    else:
                        nc.vector.tensor_scalar(on, oq_s[:, i, :],
                                                r_all[:, i:i + 1], None,
                                                OP.mult)
                    pt = pj_pool.tile([128, 512], f32, tag="pj")
                    ptv = pt.bitcast(f16)
                    nc.tensor.transpose(ptv[:, 0:128], on, ident)
                    nc.vector.tensor_copy(
                        oT_sb[:, h, qt * 128:(qt + 1) * 128], ptv[:, 0:128])

            ot_tiles = {}

            def outproj(s, mt):
                if s not in ot_tiles:
                    ot_tiles[s] = otp.tile([128, 8, 512], f16, tag="ot",
                                           name=f"ot_{s}")
                ot = ot_tiles[s]
                ps = pj_pool.tile([128, 512], f32, tag="pj")
                for kt2 in range(2):
                    nc.tensor.matmul(
                        ps[:],
                        lhsT=wo_sb[:, kt2, mt * 128:(mt + 1) * 128],
                        rhs=oT_sb[:, kt2, s * 512:(s + 1) * 512],
                        start=(kt2 == 0), stop=(kt2 == 1),
                    )
                if mt % 2 == 1:
                    nc.scalar.copy(ot[:, mt, :], ps[:])
                else:
                    nc.vector.tensor_copy(ot[:, mt, :], ps[:])
                if mt == 3:
                    nc.sync.dma_start(
                        outT_r[:, 0:4, s * 512:(s + 1) * 512], ot[:, 0:4, :])
                elif mt == 7:
                    nc.sync.dma_start(
                        outT_r[:, 4:8, s * 512:(s + 1) * 512], ot[:, 4:8, :])

            def outproj_pair(s, j):
                """Tail variant: mt pair (2j, 2j+1) in one idle scores-ring
                tile; quarter-sized output DMA per pair."""
                if s not in ot_tiles:
                    ot_tiles[s] = otp.tile([128, 8, 512], f16, tag="ot",
                                           name=f"ot_{s}")
                ot = ot_tiles[s]
                ps = sc_pool.tile([128, 1024], f32, tag="sc")
                for mi in range(2):
                    mt = 2 * j + mi
                    for kt2 in range(2):
                        nc.tensor.matmul(
                            ps[:, mi * 512:(mi + 1) * 512],
                            lhsT=wo_sb[:, kt2, mt * 128:(mt + 1) * 128],
                            rhs=oT_sb[:, kt2, s * 512:(s + 1) * 512],
                            start=(kt2 == 0), stop=(kt2 == 1),
                            skip_group_check=(mi == 1),
                        )
                for mi in range(2):
                    mt = 2 * j + mi
                    if mi == 0:
                        nc.vector.tensor_copy(ot[:, mt, :],
                                              ps[:, 0:512])
                    else:
                        nc.scalar.copy(ot[:, mt, :], ps[:, 512:1024])
                    nc.sync.dma_start(
                        outT_r[:, mt, s * 512:(s + 1) * 512], ot[:, mt, :])

            # ---------------- flat schedule ----------------
            U = []  # list of thunks

            def k_u(h, b, half=None):
                return lambda: proj_qk("wkh", "wkl", kT_sb, h, b, half)

            def q_u(h, b, half=None):
                return lambda: proj_qk("wqh", "wql", qT_sb, h, b, half)

            def v_u(st):
                return lambda: proj_v(st)

            def sc_u(s, h, kt):
                return lambda: scores(s, h, kt)

            def pv_u(s, h, i):
                return lambda: pv(s, h, i)

            def nt_u(s, h):
                return lambda: norm_tail(s, h)

            def op_u(s, mt):
                return lambda: outproj(s, mt)

            def a_phase(s, fillers, extra=None):
                fi = iter(fillers)
                for kt in range(4 * s + 4):
                    U.append(sc_u(s, 0, kt))
                    U.append(sc_u(s, 1, kt))
                    for _ in range(2):
                        f = next(fi, None)
                        if f is not None:
                            U.append(f)
                    if extra and kt in extra:
                        U.extend(extra[kt])
                rest = list(fi)
                U.extend(rest)

            U += [k_u(0, 0, 0), k_u(1, 0, 0), k_u(0, 0, 1), k_u(1, 0, 1),
                  q_u(0, 0, 0), q_u(1, 0, 0), q_u(0, 0, 1), q_u(1, 0, 1)]

            a_phase(0, [v_u(0), v_u(1), v_u(2), v_u(3),
                        k_u(0, 1), k_u(1, 1), q_u(0, 2), q_u(1, 2)])

            a_phase(2, [pv_u(0, 0, 0), pv_u(0, 1, 0),
                        pv_u(0, 0, 1), pv_u(0, 1, 1),
                        k_u(0, 2), k_u(1, 2),
                        pv_u(0, 0, 2), pv_u(0, 1, 2),
                        pv_u(0, 0, 3), pv_u(0, 1, 3),
                        nt_u(0, 0), nt_u(0, 1),
                        q_u(0, 3), q_u(1, 3),
                        v_u(4), v_u(5), v_u(6), v_u(7), v_u(8), v_u(9),
                        v_u(10), v_u(11),
                        op_u(0, 0), op_u(0, 1)])

            a_phase(3, [k_u(0, 3), k_u(1, 3),
                        op_u(0, 2), op_u(0, 3),
                        pv_u(2, 0, 0), pv_u(2, 1, 0),
                        pv_u(2, 0, 1), pv_u(2, 1, 1),
                        op_u(0, 4), op_u(0, 5),
                        pv_u(2, 0, 2), pv_u(2, 1, 2),
                        pv_u(2, 0, 3), pv_u(2, 1, 3),
                        nt_u(2, 0), nt_u(2, 1),
                        op_u(0, 6), op_u(0, 7),
                        op_u(2, 0), op_u(2, 1), op_u(2, 2), op_u(2, 3),
                        op_u(2, 4), op_u(2, 5), op_u(2, 6), op_u(2, 7),
                        q_u(0, 1), q_u(1, 1),
                        v_u(12), v_u(13), v_u(14), v_u(15),
                        pv_u(3, 0, 0), pv_u(3, 1, 0)])

            a_phase(1, [pv_u(3, 0, 1), pv_u(3, 1, 1),
                        pv_u(3, 0, 2), pv_u(3, 1, 2),
                        pv_u(3, 0, 3), pv_u(3, 1, 3),
                        nt_u(3, 0), nt_u(3, 1),
                        op_u(3, 0), op_u(3, 1), op_u(3, 2), op_u(3, 3),
                        op_u(3, 4), op_u(3, 5), op_u(3, 6), op_u(3, 7)],
                    extra={5: [pv_u(1, 0, 0)],
                           6: [pv_u(1, 0, 1)],
                           7: [pv_u(1, 0, 2)]})

            U += [pv_u(1, 0, 3),
                  nt_u(1, 0),
                  pv_u(1, 1, 0), pv_u(1, 1, 1),
                  pv_u(1, 1, 2), pv_u(1, 1, 3),
                  nt_u(1, 1),
                  (lambda: outproj_pair(1, 0)), (lambda: outproj_pair(1, 1)),
                  (lambda: outproj_pair(1, 2)), (lambda: outproj_pair(1, 3))]

            for u in U:
                u()

    nc.compile()
    return nc


def _split_fp8(a):
    import ml_dtypes
    f8 = ml_dtypes.float8_e4m3
    hi = a.astype(f8)
    lo = (a - hi.astype(np.float32)).astype(f8)
    return hi, lo


def _interleave_w(wT):
    """[D, 256] -> [128, KT*256] with arr[p, kt*256+m] = wT[kt*128+p, m]."""
    return np.ascontiguousarray(
        wT.reshape(KT, 128, 256).transpose(1, 0, 2).reshape(128, KT * 256))


def _prep_inputs(x, Wq, Wk, Wv, Wo):
    """Build the 8 per-core input maps (host-side shard/split/transpose)."""
    f16 = np.float16
    scale = HD ** -0.5
    xs = []
    for b in range(B):
        xT = np.ascontiguousarray(x[b].T).astype(np.float32)
        xs.append(_split_fp8(xT))
    in_maps = []
    for d in range(N_CORES):
        b, p = divmod(d, 4)
        r0 = 256 * p
        xh, xl = xs[b]
        wq = np.ascontiguousarray(Wq[r0:r0 + 256, :].T) * (scale * QSCALE)
        wk = np.ascontiguousarray(Wk[r0:r0 + 256, :].T) * KSCALE
        wv = np.ascontiguousarray(Wv[r0:r0 + 256, :].T) * VSCALE
        wqh, wql = _split_fp8(wq.astype(np.float32))
        wkh, wkl = _split_fp8(wk.astype(np.float32))
        wvh, wvl = _split_fp8(wv.astype(np.float32))
        in_maps.append({
            "xh": xh, "xl": xl,
            "wqh": _interleave_w(wqh), "wql": _interleave_w(wql),
            "wkh": _interleave_w(wkh), "wkl": _interleave_w(wkl),
            "wvh": _interleave_w(wvh), "wvl": _interleave_w(wvl),
            "woT": np.ascontiguousarray(Wo[:, r0:r0 + 256].T).astype(f16),
        })
    return in_maps


_CACHED = {}


def _get_program(lam: float):
    # the program depends on inputs only through lam
    key = round(float(lam), 9)
    if key not in _CACHED:
        _CACHED[key] = _build_program(float(lam))
    return _CACHED[key]


def kernel(x, Wq, Wk, Wv, Wo, lq1, lk1, lq2, lk2):
    from concourse.bass_utils import run_bass_kernel_spmd

    x = np.asarray(x, dtype=np.float32)
    Wq = np.asarray(Wq, dtype=np.float32)
    Wk = np.asarray(Wk, dtype=np.float32)
    Wv = np.asarray(Wv, dtype=np.float32)
    Wo = np.asarray(Wo, dtype=np.float32)
    lq1 = np.asarray(lq1, dtype=np.float32)
    lk1 = np.asarray(lk1, dtype=np.float32)
    lq2 = np.asarray(lq2, dtype=np.float32)
    lk2 = np.asarray(lk2, dtype=np.float32)

    lam1 = np.exp(np.sum(lq1 * lk1, dtype=np.float32))
    lam2 = np.exp(np.sum(lq2 * lk2, dtype=np.float32))
    lam = float(lam1 - lam2 + LAMBDA_INIT)

    nc = _get_program(lam)
    in_maps = _prep_inputs(x, Wq, Wk, Wv, Wo)
    res = run_bass_kernel_spmd(nc, in_maps, core_ids=list(range(N_CORES)))

    out = np.empty((B, S, D), dtype=np.float32)
    for b in range(B):
        acc = res.results[4 * b]["outT"].astype(np.float32)
        for p in range(1, 4):
            acc += res.results[4 * b + p]["outT"].astype(np.float32)
        out[b] = acc.T
    return out


# revision 45
# speedup vs baseline: 1.2057x; 1.0246x over previous
"""DiffAttention Trainium2 kernel (fp8 hi/lo projections + pipelined schedule).

Full inputs in, full output out. Sharding: 8 cores = (batch b in {0,1}) x
(head-pair p in {0..3}); each core handles one batch element and 2 of the 8
heads (= 4 of the 16 q/k half-heads, 2 v heads, 256 of the 1024 o columns).
Out-projection is column-split: each core produces a full (S, D) partial of
o @ Wo.T restricted to its o columns; host sums the 4 partials per batch.

Projections run as fp8e4m3 DoubleRow matmuls with host-side error
compensation: x and each W shard are split hi/lo (hi = fp8(t), lo =
fp8(t - hi)) and the three significant products xh@Wh + xh@Wl + xl@Wh are
accumulated in PSUM (the dropped xl@Wl term is ~0.07% relative).  DoubleRow
contracts two 128-deep k-tiles per instruction at 0.5 cycles/row, so each
projection costs 6 rows/out-tile instead of fp16's 8.  Weights carry
power-of-2 pre-scales (q: 2^7*hd^-0.5, k/v: 2^4) to center fp8 exponents;
the combined 2^-11 is folded into the exp's scale argument and the RMSNorm
epsilon (the norm itself is scale-invariant), so no evacuation rescale is
needed anywhere.

Attention math per head h (half-heads e0=2h, e1=2h+1), per q row:
  u_i = exp(s_i) @ v   (unnormalized), sum_i = exp(s_i) @ 1  (fused: rhs=[v|1])
  o   = u0/sum0 - lam * u1/sum1
  o   = o * rsqrt(mean(o^2)+eps) * (1-lam_init);   out = o @ Wo.T
Scores are computed transposed (keys on partitions, q on free dim) so the
exp'd tiles feed the PV matmul directly as the stationary operand.  rsqrt is
Newton-Raphson on the DVE (fast-inverse-sqrt seed), batched per (strip, head).

Scheduling: a single flat unit stream, strip order [0,2,3,1] (small strip
last to shrink the drain tail).  Each strip's score+exp units are
interleaved with filler PE work (previous strip's PV chains / norm tails /
out-projection chunks, projection units, v-projection tiles) so the PE never
waits on the ACT exp pipeline and never idles long enough to drop out of its
high p-state.  Input DMA is chunked (x in 512-column blocks — 512-byte
per-partition runs, the minimum for full DMA descriptor rate; weights
hi-before-lo) and ordered by first use.  Evacuations are balanced across
DVE/ACT (GPSIMD cannot read PSUM); the tail strip's out-projection uses the
by-then-idle scores PSUM ring in [128,1024] mt-pairs to halve its
matmul/evac ladder, with 'on'-scaling on ACT.  PSUM: scores 2x[128,1024] +
PV 2x[128,258] + a shared [128,512] ring (projection evac / out-projection /
transposes, which stay at 1 cycle/row because the identity operand is f16)
= exactly 8 banks.
"""

import math

import numpy as np

B = 2
S = 2048
D = 1024
H = 8
HD = 64  # half-head dim
LAMBDA_INIT = 0.8 - 0.6 * math.exp(-0.3 * 6)
EPS = 1e-5

N_CORES = 8
KT = D // 128      # 8 contraction tiles for projections
ST = S // 128      # 16 sequence tiles
NSTRIP = S // 512  # 4 q strips

QSCALE = 2.0 ** 7   # folded into WqT (on top of hd^-0.5)
KSCALE = 2.0 ** 4   # folded into WkT
VSCALE = 2.0 ** 4   # folded into WvT
SSCALE = 1.0 / (QSCALE * KSCALE)   # exp() input scale
OSCALE2 = float(VSCALE * VSCALE)   # o is VSCALE-scaled; ss is VSCALE^2-scaled


def _build_program(lam: float):
    import concourse.bass as bass
    import concourse.tile as tile
    from concourse import bacc, mybir
    from concourse.masks import make_identity

    f8 = mybir.dt.float8e4
    f16 = mybir.dt.float16
    f32 = mybir.dt.float32
    u32 = mybir.dt.uint32
    AF = mybir.ActivationFunctionType
    OP = mybir.AluOpType
    DR = mybir.MatmulPerfMode.DoubleRow

    nc = bacc.Bacc("TRN2", target_bir_lowering=False, debug=False,
                   num_devices=N_CORES)

    xh_d = nc.dram_tensor("xh", (D, S), f8, kind="ExternalInput").ap()
    xl_d = nc.dram_tensor("xl", (D, S), f8, kind="ExternalInput").ap()
    # weights host-interleaved to [128, KT*256] for contiguous 2KB DMA runs
    w_d = {}
    for nm in ("wqh", "wql", "wkh", "wkl", "wvh", "wvl"):
        w_d[nm] = nc.dram_tensor(nm, (128, KT * 256), f8,
                                 kind="ExternalInput").ap()
    woT = nc.dram_tensor("woT", (256, D), f16, kind="ExternalInput").ap()
    outT = nc.dram_tensor("outT", (D, S), f16, kind="ExternalOutput").ap()

    with tile.TileContext(nc) as tc:
        with (
            tc.tile_pool(name="const", bufs=1) as cpool,
            tc.tile_pool(name="persist", bufs=1) as pp,
            tc.tile_pool(name="pin", bufs=1) as pin,
            tc.tile_pool(name="e0p", bufs=24) as e0pool,
            tc.tile_pool(name="e1p", bufs=24) as e1pool,
            tc.tile_pool(name="sc", bufs=2, space="PSUM") as sc_pool,
            tc.tile_pool(name="up", bufs=2, space="PSUM") as up_pool,
            tc.tile_pool(name="pj", bufs=2, space="PSUM") as pj_pool,
            tc.tile_pool(name="nrm", bufs=8) as nrm,
            tc.tile_pool(name="nrm_big", bufs=2) as nrm_big,
            tc.tile_pool(name="osb", bufs=3) as osb,
            tc.tile_pool(name="otp", bufs=2) as otp,
        ):
            epools = {0: e0pool, 1: e1pool}

            ident = cpool.tile([128, 128], f16, tag="ident")
            make_identity(nc, ident)
            # mask[p, f] = 1 if p <= f else 0 (keys on partitions, q on free)
            maskt = cpool.tile([128, 128], f16, tag="maskt")
            nc.gpsimd.memset(maskt, 1.0)
            nc.gpsimd.affine_select(
                out=maskt, in_=maskt, compare_op=OP.is_ge, fill=0.0,
                base=0, pattern=[[1, 128]], channel_multiplier=-1,
            )
            # constants for Newton-Raphson rsqrt (fast-inverse-sqrt seed)
            magic_c = cpool.tile([128, 8], u32, tag="magic_c")
            nc.gpsimd.memset(magic_c, 0x5F3759DF)
            one_u = cpool.tile([128, 8], u32, tag="one_u")
            nc.gpsimd.memset(one_u, 1)

            qT_sb = pp.tile([128, 2, S], f16, tag="qT_sb")
            kT_sb = pp.tile([128, 2, S], f16, tag="kT_sb")
            v_sb = pp.tile([128, ST, 258], f16, tag="v_sb")
            nc.vector.memset(v_sb[:, :, 128:129], 1.0)
            nc.vector.memset(v_sb[:, :, 257:258], 1.0)
            oT_sb = pp.tile([128, 2, S], f16, tag="oT_sb")
            wo_sb = pp.tile([128, 2, D], f16, tag="wo_sb")

            xh_sb = pin.tile([128, KT, S], f8, tag="xh_sb")
            xl_sb = pin.tile([128, KT, S], f8, tag="xl_sb")
            w_sb = {}
            for nm in ("wqh", "wql", "wkh", "wkl", "wvh", "wvl"):
                wt = pin.tile([128, KT, 256], f8, tag=nm + "_sb",
                              name=nm + "_sb")
                w_sb[nm] = wt

            # ---------------- input DMAs, ordered by first use -------------
            xh_r = xh_d.rearrange("(kt p) s -> p kt s", p=128)
            xl_r = xl_d.rearrange("(kt p) s -> p kt s", p=128)

            def dma_w(nm):
                nc.sync.dma_start(
                    w_sb[nm][:, :, :],
                    w_d[nm].rearrange("p (kt m) -> p kt m", m=256)[:, :, :])

            def dma_x(b):
                c0, c1 = 512 * b, 512 * (b + 1)
                nc.sync.dma_start(xh_sb[:, :, c0:c1], xh_r[:, :, c0:c1])
                nc.sync.dma_start(xl_sb[:, :, c0:c1], xl_r[:, :, c0:c1])

            def dma_x_half(b, half):
                c0 = 512 * b + 256 * half
                c1 = c0 + 256
                nc.sync.dma_start(xh_sb[:, :, c0:c1], xh_r[:, :, c0:c1])
                nc.sync.dma_start(xl_sb[:, :, c0:c1], xl_r[:, :, c0:c1])

            dma_w("wkh")
            dma_x(0)
            dma_w("wkl")
            dma_w("wqh")
            dma_w("wql")
            dma_w("wvh")
            dma_w("wvl")
            dma_x(1)
            dma_x(2)
            nc.gpsimd.dma_start(
                wo_sb[:, :, :],
                woT.rearrange("(kt p) n -> p kt n", p=128)[:, :, :])
            dma_x(3)

            # ---------------- unit closures ----------------
            e_tiles = {}     # (s, h, kt) -> SBUF tile [128, 1024] f16
            oq_tiles = {}    # (s, h) -> [128, 4, 128] f32
            ss_tiles = {}    # (s, h) -> [128, 4] f32
            outT_r = outT.rearrange("(mt p) s -> p mt s", p=128)

            def proj_qk(wh, wl, dst_sb, h, b, half=None):
                """qT/kT columns [b*512,(b+1)*512) for local head h."""
                ps = pj_pool.tile([128, 512], f32, tag="pj")
                if half is None:
                    c0, c1, w0 = 512 * b, 512 * (b + 1), 512
                else:
                    c0 = 512 * b + 256 * half
                    c1, w0 = c0 + 256, 256
                m0, m1 = 128 * h, 128 * (h + 1)
                chains = ((wh, xh_sb), (wh, xl_sb), (wl, xh_sb))
                for ci, (w, x) in enumerate(chains):
                    for pr in range(KT // 2):
                        nc.tensor.matmul(
                            ps[:, 0:w0],
                            lhsT=w_sb[w][:, 2 * pr:2 * pr + 2, m0:m1],
                            rhs=x[:, 2 * pr:2 * pr + 2, c0:c1],
                            start=(ci == 0 and pr == 0),
                            stop=(ci == 2 and pr == KT // 2 - 1),
                            perf_mode=DR,
                        )
                nc.vector.tensor_copy(dst_sb[:, h, c0:c1], ps[:, 0:w0])

            def proj_v(st):
                """v rows [st*128,(st+1)*128) for both local heads."""
                ps = pj_pool.tile([128, 512], f32, tag="pj")
                r0, r1 = 128 * st, 128 * (st + 1)
                chains = (("wvh", xh_sb), ("wvh", xl_sb), ("wvl", xh_sb))
                for ci, (w, x) in enumerate(chains):
                    for pr in range(KT // 2):
                        nc.tensor.matmul(
                            ps[:, 0:256],
                            lhsT=x[:, 2 * pr:2 * pr + 2, r0:r1],
                            rhs=w_sb[w][:, 2 * pr:2 * pr + 2, :],
                            start=(ci == 0 and pr == 0),
                            stop=(ci == 2 and pr == KT // 2 - 1),
                            perf_mode=DR,
                        )
                nc.vector.tensor_copy(v_sb[:, st, 0:128], ps[:, 0:128])
                nc.vector.tensor_copy(v_sb[:, st, 129:257], ps[:, 128:256])

            def scores(s, h, kt):
                col0 = max(0, (kt - 4 * s) * 128)
                pa = sc_pool.tile([128, 1024], f32, tag="sc")
                nc.tensor.matmul(
                    pa[:, col0:512],
                    lhsT=kT_sb[0:64, h, kt * 128:(kt + 1) * 128],
                    rhs=qT_sb[0:64, h, s * 512 + col0:(s + 1) * 512],
                    start=True, stop=True, tile_position=(0, 0),
                )
                nc.tensor.matmul(
                    pa[:, 512 + col0:1024],
                    lhsT=kT_sb[64:128, h, kt * 128:(kt + 1) * 128],
                    rhs=qT_sb[64:128, h, s * 512 + col0:(s + 1) * 512],
                    start=True, stop=True, tile_position=(64, 0),
                    skip_group_check=True,
                )
                ee = epools[h].tile([128, 1024], f16, tag="e")
                nc.scalar.activation(
                    ee.rearrange("p (b c) -> p b c", b=2)[:, :, col0:512],
                    pa.rearrange("p (b c) -> p b c", b=2)[:, :, col0:512],
                    AF.Exp, scale=SSCALE)
                if kt >= 4 * s:
                    c = col0
                    nc.gpsimd.tensor_mul(ee[:, c:c + 128],
                                         ee[:, c:c + 128], maskt)
                    nc.gpsimd.tensor_mul(ee[:, 512 + c:512 + c + 128],
                                         ee[:, 512 + c:512 + c + 128],
                                         maskt)
                e_tiles[(s, h, kt)] = ee

            def pv(s, h, i):
                """PV chain + per-qtile normalized diff for qtile i of strip s."""
                qt = 4 * s + i
                if (s, h) not in oq_tiles:
                    oq_tiles[(s, h)] = nrm_big.tile(
                        [128, 4, 128], f16, tag="oq_s", name=f"oq_{s}_{h}")
                    ss_tiles[(s, h)] = nrm_big.tile(
                        [128, 4], f32, tag="ss_s", name=f"ss_{s}_{h}")
                oq_s = oq_tiles[(s, h)]
                ss_s = ss_tiles[(s, h)]
                up = up_pool.tile([128, 258], f32, tag="up")
                c = i * 128
                for kt in range(qt + 1):
                    ee = e_tiles[(s, h, kt)]
                    vh = v_sb[:, kt, 129 * h:129 * h + 129]
                    nc.tensor.matmul(
                        up[:, 0:129],
                        lhsT=ee[:, c:c + 128],
                        rhs=vh,
                        start=(kt == 0), stop=(kt == qt),
                    )
                    nc.tensor.matmul(
                        up[:, 129:258],
                        lhsT=ee[:, 512 + c:512 + c + 128],
                        rhs=vh,
                        start=False, stop=(kt == qt),
                        skip_group_check=True,
                    )
                inv0 = nrm.tile([128, 1], f32, tag="inv0")
                nc.vector.reciprocal(inv0, up[:, 128:129])
                inv1 = nrm.tile([128, 1], f32, tag="inv1")
                nc.vector.reciprocal(inv1, up[:, 257:258])
                t1 = nrm.tile([128, 128], f32, tag="t1")
                nc.vector.tensor_scalar(t1, up[:, 129:257], inv1, lam,
                                        OP.mult, OP.mult)
                oq = oq_s[:, i, :]
                nc.vector.scalar_tensor_tensor(
                    oq, up[:, 0:128], inv0, t1, OP.mult, OP.subtract)
                sq = nrm.tile([128, 128], f16, tag="sq")
                nc.vector.scalar_tensor_tensor(
                    sq, oq, 1.0, oq, OP.bypass, OP.mult,
                    accum_out=ss_s[:, i:i + 1])

            def norm_tail(s, h):
                """Newton rsqrt over the strip's 4 q-tiles, scale, transpose."""
                oq_s = oq_tiles[(s, h)]
                ss_s = ss_tiles[(s, h)]
                ms = nrm.tile([128, 4], f32, tag="ms")
                il2 = 1.0 / ((1.0 - LAMBDA_INIT) ** 2)
                nc.vector.tensor_scalar(ms, ss_s, il2 / 128.0,
                                        EPS * il2 * OSCALE2,
                                        OP.mult, OP.add)
                y0 = nrm.tile([128, 4], u32, tag="y0")
                nc.vector.tensor_tensor(y0, ms.bitcast(u32), one_u[:, 0:4],
                                        OP.logical_shift_right)
                nc.vector.tensor_tensor(y0, magic_c[:, 0:4], y0, OP.subtract)
                yf = y0.bitcast(f32)
                t2 = nrm.tile([128, 4], f32, tag="t2")
                r_all = nrm.tile([128, 4], f32, tag="r_all")
                nc.vector.tensor_mul(t2, yf, yf)
                nc.vector.tensor_mul(t2, t2, ms)
                nc.vector.tensor_scalar(t2, t2, -0.5, 1.5, OP.mult, OP.add)
                nc.vector.tensor_mul(r_all, yf, t2)
                nc.vector.tensor_mul(t2, r_all, r_all)
                nc.vector.tensor_mul(t2, t2, ms)
                nc.vector.tensor_scalar(t2, t2, -0.5, 1.5, OP.mult, OP.add)
                nc.vector.tensor_mul(r_all, r_all, t2)
                for i in range(4):
                    qt = 4 * s + i
                    on = osb.tile([128, 128], f16, tag="on")
                    if s == 1:
                        # tail strip: ACT is exp-idle by now; keep DVE clear
                        nc.scalar.activation(on, oq_s[:, i, :], AF.Copy,
                                             scale=r_all[:, i:i + 1])
                    else:
                        nc.vector.tensor_scalar(on, oq_s[:, i, :],
                                                r_all[:, i:i + 1], None,
                                                OP.mult)
                    pt = pj_pool.tile([128, 512], f32, tag="pj")
                    ptv = pt.bitcast(f16)
                    nc.tensor.transpose(ptv[:, 0:128], on, ident)
                    nc.vector.tensor_copy(
                        oT_sb[:, h, qt * 128:(qt + 1) * 128], ptv[:, 0:128])

            ot_tiles = {}

            def outproj(s, mt):
                if s not in ot_tiles:
                    ot_tiles[s] = otp.tile([128, 8, 512], f16, tag="ot",
                                           name=f"ot_{s}")
                ot = ot_tiles[s]
                ps = pj_pool.tile([128, 512], f32, tag="pj")
                for kt2 in range(2):
                    nc.tensor.matmul(
                        ps[:],
                        lhsT=wo_sb[:, kt2, mt * 128:(mt + 1) * 128],
                        rhs=oT_sb[:, kt2, s * 512:(s + 1) * 512],
                        start=(kt2 == 0), stop=(kt2 == 1),
                    )
                if mt % 2 == 1:
                    nc.scalar.copy(ot[:, mt, :], ps[:])
                else:
                    nc.vector.tensor_copy(ot[:, mt, :], ps[:])
                if mt == 3:
                    nc.sync.dma_start(
                        outT_r[:, 0:4, s * 512:(s + 1) * 512], ot[:, 0:4, :])
                elif mt == 7:
                    nc.sync.dma_start(
                        outT_r[:, 4:8, s * 512:(s + 1) * 512], ot[:, 4:8, :])

            def outproj_pair(s, j):
                """Tail variant: mt pair (2j, 2j+1) in one idle scores-ring
                tile; quarter-sized output DMA per pair."""
                if s not in ot_tiles:
                    ot_tiles[s] = otp.tile([128, 8, 512], f16, tag="ot",
                                           name=f"ot_{s}")
                ot = ot_tiles[s]
                ps = sc_pool.tile([128, 1024], f32, tag="sc")
                for mi in range(2):
                    mt = 2 * j + mi
                    for kt2 in range(2):
                        nc.tensor.matmul(
                            ps[:, mi * 512:(mi + 1) * 512],
                            lhsT=wo_sb[:, kt2, mt * 128:(mt + 1) * 128],
                            rhs=oT_sb[:, kt2, s * 512:(s + 1) * 512],
                            start=(kt2 == 0), stop=(kt2 == 1),
                            skip_group_check=(mi == 1),
                        )
                for mi in range(2):
                    mt = 2 * j + mi
                    if mi == 0:
                        nc.vector.tensor_copy(ot[:, mt, :],
                                              ps[:, 0:512])
                    else:
                        nc.scalar.copy(ot[:, mt, :], ps[:, 512:1024])
                    nc.sync.dma_start(
                        outT_r[:, mt, s * 512:(s + 1) * 512], ot[:, mt, :])

            # ---------------- flat schedule ----------------
            U = []  # list of thunks

            def k_u(h, b, half=None):
                return lambda: proj_qk("wkh", "wkl", kT_sb, h, b, half)

            def q_u(h, b, half=None):
                return lambda: proj_qk("wqh", "wql", qT_sb, h, b, half)

            def v_u(st):
                return lambda: proj_v(st)

            def sc_u(s, h, kt):
                return lambda: scores(s, h, kt)

            def pv_u(s, h, i):
                return lambda: pv(s, h, i)

            def nt_u(s, h):
                return lambda: norm_tail(s, h)

            def op_u(s, mt):
                return lambda: outproj(s, mt)

            def a_phase(s, fillers, extra=None):
                fi = iter(fillers)
                for kt in range(4 * s + 4):
                    U.append(sc_u(s, 0, kt))
                    U.append(sc_u(s, 1, kt))
                    for _ in range(2):
                        f = next(fi, None)
                        if f is not None:
                            U.append(f)
                    if extra and kt in extra:
                        U.extend(extra[kt])
                rest = list(fi)
                U.extend(rest)

            U += [k_u(0, 0), k_u(1, 0), q_u(0, 0), q_u(1, 0)]

            a_phase(0, [v_u(0), v_u(1), v_u(2), v_u(3),
                        k_u(0, 1), k_u(1, 1), q_u(0, 2), q_u(1, 2)])

            a_phase(2, [pv_u(0, 0, 0), pv_u(0, 1, 0),
                        pv_u(0, 0, 1), pv_u(0, 1, 1),
                        k_u(0, 2), k_u(1, 2),
                        pv_u(0, 0, 2), pv_u(0, 1, 2),
                        pv_u(0, 0, 3), pv_u(0, 1, 3),
                        nt_u(0, 0), nt_u(0, 1),
                        q_u(0, 3), q_u(1, 3),
                        v_u(4), v_u(5), v_u(6), v_u(7), v_u(8), v_u(9),
                        v_u(10), v_u(11),
                        op_u(0, 0), op_u(0, 1)])

            a_phase(3, [k_u(0, 3), k_u(1, 3),
                        op_u(0, 2), op_u(0, 3),
                        pv_u(2, 0, 0), pv_u(2, 1, 0),
                        pv_u(2, 0, 1), pv_u(2, 1, 1),
                        op_u(0, 4), op_u(0, 5),
                        pv_u(2, 0, 2), pv_u(2, 1, 2),
                        pv_u(2, 0, 3), pv_u(2, 1, 3),
                        nt_u(2, 0), nt_u(2, 1),
                        op_u(0, 6), op_u(0, 7),
                        op_u(2, 0), op_u(2, 1), op_u(2, 2), op_u(2, 3),
                        op_u(2, 4), op_u(2, 5), op_u(2, 6), op_u(2, 7),
                        q_u(0, 1), q_u(1, 1),
                        v_u(12), v_u(13), v_u(14), v_u(15),
                        pv_u(3, 0, 0), pv_u(3, 1, 0)])

            a_phase(1, [pv_u(3, 0, 1), pv_u(3, 1, 1),
                        pv_u(3, 0, 2), pv_u(3, 1, 2),
                        pv_u(3, 0, 3), pv_u(3, 1, 3),
                        nt_u(3, 0), nt_u(3, 1),
                        op_u(3, 0), op_u(3, 1), op_u(3, 2), op_u(3, 3),
                        op_u(3, 4), op_u(3, 5), op_u(3, 6), op_u(3, 7)],
                    extra={5: [pv_u(1, 0, 0)],
                           6: [pv_u(1, 0, 1)],
                           7: [pv_u(1, 0, 2)]})

            U += [pv_u(1, 0, 3),
                  nt_u(1, 0),
                  pv_u(1, 1, 0), pv_u(1, 1, 1),
                  pv_u(1, 1, 2), pv_u(1, 1, 3),
                  nt_u(1, 1),
                  (lambda: outproj_pair(1, 0)), (lambda: outproj_pair(1, 1)),
                  (lambda: outproj_pair(1, 2)), (lambda: outproj_pair(1, 3))]

            for u in U:
                u()

    nc.compile()
    return nc


def _split_fp8(a):
    import ml_dtypes
    f8 = ml_dtypes.float8_e4m3
    hi = a.astype(f8)
    lo = (a - hi.astype(np.float32)).astype(f8)
    return hi, lo


def _interleave_w(wT):
    """[D, 256] -> [128, KT*256] with arr[p, kt*256+m] = wT[kt*128+p, m]."""
    return np.ascontiguousarray(
        wT.reshape(KT, 128, 256).transpose(1, 0, 2).reshape(128, KT * 256))


def _prep_inputs(x, Wq, Wk, Wv, Wo):
    """Build the 8 per-core input maps (host-side shard/split/transpose)."""
    f16 = np.float16
    scale = HD ** -0.5
    xs = []
    for b in range(B):
        xT = np.ascontiguousarray(x[b].T).astype(np.float32)
        xs.append(_split_fp8(xT))
    in_maps = []
    for d in range(N_CORES):
        b, p = divmod(d, 4)
        r0 = 256 * p
        xh, xl = xs[b]
        wq = np.ascontiguousarray(Wq[r0:r0 + 256, :].T) * (scale * QSCALE)
        wk = np.ascontiguousarray(Wk[r0:r0 + 256, :].T) * KSCALE
        wv = np.ascontiguousarray(Wv[r0:r0 + 256, :].T) * VSCALE
        wqh, wql = _split_fp8(wq.astype(np.float32))
        wkh, wkl = _split_fp8(wk.astype(np.float32))
        wvh, wvl = _split_fp8(wv.astype(np.float32))
        in_maps.append({
            "xh": xh, "xl": xl,
            "wqh": _interleave_w(wqh), "wql": _interleave_w(wql),
            "wkh": _interleave_w(wkh), "wkl": _interleave_w(wkl),
            "wvh": _interleave_w(wvh), "wvl": _interleave_w(wvl),
            "woT": np.ascontiguousarray(Wo[:, r0:r0 + 256].T).astype(f16),
        })
    return in_maps


_CACHED = {}


def _get_program(lam: float):
    # the program depends on inputs only through lam
    key = round(float(lam), 9)
    if key not in _CACHED:
        _CACHED[key] = _build_program(float(lam))
    return _CACHED[key]


def kernel(x, Wq, Wk, Wv, Wo, lq1, lk1, lq2, lk2):
    from concourse.bass_utils import run_bass_kernel_spmd

    x = np.asarray(x, dtype=np.float32)
    Wq = np.asarray(Wq, dtype=np.float32)
    Wk = np.asarray(Wk, dtype=np.float32)
    Wv = np.asarray(Wv, dtype=np.float32)
    Wo = np.asarray(Wo, dtype=np.float32)
    lq1 = np.asarray(lq1, dtype=np.float32)
    lk1 = np.asarray(lk1, dtype=np.float32)
    lq2 = np.asarray(lq2, dtype=np.float32)
    lk2 = np.asarray(lk2, dtype=np.float32)

    lam1 = np.exp(np.sum(lq1 * lk1, dtype=np.float32))
    lam2 = np.exp(np.sum(lq2 * lk2, dtype=np.float32))
    lam = float(lam1 - lam2 + LAMBDA_INIT)

    nc = _get_program(lam)
    in_maps = _prep_inputs(x, Wq, Wk, Wv, Wo)
    res = run_bass_kernel_spmd(nc, in_maps, core_ids=list(range(N_CORES)))

    out = np.empty((B, S, D), dtype=np.float32)
    for b in range(B):
        acc = res.results[4 * b]["outT"].astype(np.float32)
        for p in range(1, 4):
            acc += res.results[4 * b + p]["outT"].astype(np.float32)
        out[b] = acc.T
    return out


# revision 67
# speedup vs baseline: 1.2342x; 1.0236x over previous
"""DiffAttention Trainium2 kernel (fp8 hi/lo projections + pipelined schedule).

Full inputs in, full output out. Sharding: 8 cores = (batch b in {0,1}) x
(head-pair p in {0..3}); each core handles one batch element and 2 of the 8
heads (= 4 of the 16 q/k half-heads, 2 v heads, 256 of the 1024 o columns).
Out-projection is column-split: each core produces a full (S, D) partial of
o @ Wo.T restricted to its o columns; host sums the 4 partials per batch.

Projections run as fp8e4m3 DoubleRow matmuls with host-side error
compensation: x and each W shard are split hi/lo (hi = fp8(t), lo =
fp8(t - hi)) and the three significant products xh@Wh + xh@Wl + xl@Wh are
accumulated in PSUM (the dropped xl@Wl term is ~0.07% relative).  DoubleRow
contracts two 128-deep k-tiles per instruction at 0.5 cycles/row, so each
projection costs 6 rows/out-tile instead of fp16's 8.  Weights carry
power-of-2 pre-scales (q: 2^7*hd^-0.5, k/v: 2^4) to center fp8 exponents;
the combined 2^-11 is folded into the exp's scale argument and the RMSNorm
epsilon (the norm itself is scale-invariant), so no evacuation rescale is
needed anywhere.

Attention math per head h (half-heads e0=2h, e1=2h+1), per q row:
  u_i = exp(s_i) @ v   (unnormalized), sum_i = exp(s_i) @ 1  (fused: rhs=[v|1])
  o   = u0/sum0 - lam * u1/sum1
  o   = o * rsqrt(mean(o^2)+eps) * (1-lam_init);   out = o @ Wo.T
Scores are computed transposed (keys on partitions, q on free dim) so the
exp'd tiles feed the PV matmul directly as the stationary operand.  rsqrt is
Newton-Raphson on the DVE (fast-inverse-sqrt seed), batched per (strip, head).

Scheduling: a single flat unit stream, strip order [0,2,3,1] (small strip
last to shrink the drain tail).  Each strip's score+exp units are
interleaved with filler PE work (previous strip's PV chains / norm tails /
out-projection chunks, projection units, v-projection tiles) so the PE never
waits on the ACT exp pipeline and never idles long enough to drop out of its
high p-state.  Input DMA is chunked (x in 512-column blocks — 512-byte
per-partition runs, the minimum for full DMA descriptor rate; weights
hi-before-lo) and ordered by first use, with xh on the SP trigger queue and
xl on the ACT queue so the two streams transfer concurrently.  Evacuations
are balanced across DVE/ACT (GPSIMD cannot read PSUM); the tail strip's
out-projection spreads over four PSUM slots (two idle scores-ring
[128,1024] pairs + two [128,512] singles) with evacs split DVE/ACT in
parallel and per-mt output DMAs on alternating SP/GPSIMD queues, shortening
the final matmul->evac->DMA drain.  PSUM: scores 2x[128,1024] + PV
2x[128,258] + a shared [128,512] ring (projection evac / out-projection /
transposes, which stay at 1 cycle/row because the identity operand is f16)
= exactly 8 banks.
"""

import math

import numpy as np

B = 2
S = 2048
D = 1024
H = 8
HD = 64  # half-head dim
LAMBDA_INIT = 0.8 - 0.6 * math.exp(-0.3 * 6)
EPS = 1e-5

N_CORES = 8
KT = D // 128      # 8 contraction tiles for projections
ST = S // 128      # 16 sequence tiles
NSTRIP = S // 512  # 4 q strips

QSCALE = 2.0 ** 7   # folded into WqT (on top of hd^-0.5)
KSCALE = 2.0 ** 4   # folded into WkT
VSCALE = 2.0 ** 4   # folded into WvT
SSCALE = 1.0 / (QSCALE * KSCALE)   # exp() input scale
OSCALE2 = float(VSCALE * VSCALE)   # o is VSCALE-scaled; ss is VSCALE^2-scaled


def _build_program(lam: float):
    import concourse.bass as bass
    import concourse.tile as tile
    from concourse import bacc, mybir
    from concourse.masks import make_identity

    f8 = mybir.dt.float8e4
    f16 = mybir.dt.float16
    f32 = mybir.dt.float32
    u32 = mybir.dt.uint32
    AF = mybir.ActivationFunctionType
    OP = mybir.AluOpType
    DR = mybir.MatmulPerfMode.DoubleRow

    nc = bacc.Bacc("TRN2", target_bir_lowering=False, debug=False,
                   num_devices=N_CORES)

    xh_d = nc.dram_tensor("xh", (D, S), f8, kind="ExternalInput").ap()
    xl_d = nc.dram_tensor("xl", (D, S), f8, kind="ExternalInput").ap()
    # weights host-interleaved to [128, KT*256] for contiguous 2KB DMA runs
    w_d = {}
    for nm in ("wqh", "wql", "wkh", "wkl", "wvh", "wvl"):
        w_d[nm] = nc.dram_tensor(nm, (128, KT * 256), f8,
                                 kind="ExternalInput").ap()
    woT = nc.dram_tensor("woT", (256, D), f16, kind="ExternalInput").ap()
    outT = nc.dram_tensor("outT", (D, S), f16, kind="ExternalOutput").ap()

    with tile.TileContext(nc) as tc:
        with (
            tc.tile_pool(name="const", bufs=1) as cpool,
            tc.tile_pool(name="persist", bufs=1) as pp,
            tc.tile_pool(name="pin", bufs=1) as pin,
            tc.tile_pool(name="e0p", bufs=24) as e0pool,
            tc.tile_pool(name="e1p", bufs=24) as e1pool,
            tc.tile_pool(name="sc", bufs=2, space="PSUM") as sc_pool,
            tc.tile_pool(name="up", bufs=2, space="PSUM") as up_pool,
            tc.tile_pool(name="pj", bufs=2, space="PSUM") as pj_pool,
            tc.tile_pool(name="nrm", bufs=8) as nrm,
            tc.tile_pool(name="nrm_big", bufs=2) as nrm_big,
            tc.tile_pool(name="osb", bufs=3) as osb,
            tc.tile_pool(name="otp", bufs=2) as otp,
        ):
            epools = {0: e0pool, 1: e1pool}

            ident = cpool.tile([128, 128], f16, tag="ident")
            make_identity(nc, ident)
            # mask[p, f] = 1 if p <= f else 0 (keys on partitions, q on free)
            maskt = cpool.tile([128, 128], f16, tag="maskt")
            nc.gpsimd.memset(maskt, 1.0)
            nc.gpsimd.affine_select(
                out=maskt, in_=maskt, compare_op=OP.is_ge, fill=0.0,
                base=0, pattern=[[1, 128]], channel_multiplier=-1,
            )
            # constants for Newton-Raphson rsqrt (fast-inverse-sqrt seed)
            magic_c = cpool.tile([128, 8], u32, tag="magic_c")
            nc.gpsimd.memset(magic_c, 0x5F3759DF)
            one_u = cpool.tile([128, 8], u32, tag="one_u")
            nc.gpsimd.memset(one_u, 1)

            qT_sb = pp.tile([128, 2, S], f16, tag="qT_sb")
            kT_sb = pp.tile([128, 2, S], f16, tag="kT_sb")
            v_sb = pp.tile([128, ST, 258], f16, tag="v_sb")
            nc.vector.memset(v_sb[:, :, 128:129], 1.0)
            nc.vector.memset(v_sb[:, :, 257:258], 1.0)
            oT_sb = pp.tile([128, 2, S], f16, tag="oT_sb")
            wo_sb = pp.tile([128, 2, D], f16, tag="wo_sb")

            xh_sb = pin.tile([128, KT, S], f8, tag="xh_sb")
            xl_sb = pin.tile([128, KT, S], f8, tag="xl_sb")
            w_sb = {}
            for nm in ("wqh", "wql", "wkh", "wkl", "wvh", "wvl"):
                wt = pin.tile([128, KT, 256], f8, tag=nm + "_sb",
                              name=nm + "_sb")
                w_sb[nm] = wt

            # ---------------- input DMAs, ordered by first use -------------
            xh_r = xh_d.rearrange("(kt p) s -> p kt s", p=128)
            xl_r = xl_d.rearrange("(kt p) s -> p kt s", p=128)

            def dma_w(nm, eng=None):
                (eng or nc.sync).dma_start(
                    w_sb[nm][:, :, :],
                    w_d[nm].rearrange("p (kt m) -> p kt m", m=256)[:, :, :])

            def dma_x(b):
                c0, c1 = 512 * b, 512 * (b + 1)
                nc.sync.dma_start(xh_sb[:, :, c0:c1], xh_r[:, :, c0:c1])
                nc.scalar.dma_start(xl_sb[:, :, c0:c1], xl_r[:, :, c0:c1])

            def dma_x_half(b, half):
                c0 = 512 * b + 256 * half
                c1 = c0 + 256
                nc.sync.dma_start(xh_sb[:, :, c0:c1], xh_r[:, :, c0:c1])
                nc.sync.dma_start(xl_sb[:, :, c0:c1], xl_r[:, :, c0:c1])

            dma_w("wkh")
            dma_x(0)
            dma_w("wkl")
            dma_w("wqh")
            dma_w("wql", nc.gpsimd)
            dma_w("wvh", nc.gpsimd)
            dma_w("wvl", nc.gpsimd)
            dma_x(1)
            dma_x(2)
            nc.gpsimd.dma_start(
                wo_sb[:, :, :],
                woT.rearrange("(kt p) n -> p kt n", p=128)[:, :, :])
            dma_x(3)

            # ---------------- unit closures ----------------
            e_tiles = {}     # (s, h, kt) -> SBUF tile [128, 1024] f16
            oq_tiles = {}    # (s, h) -> [128, 4, 128] f32
            ss_tiles = {}    # (s, h) -> [128, 4] f32
            outT_r = outT.rearrange("(mt p) s -> p mt s", p=128)

            def proj_qk(wh, wl, dst_sb, h, b, half=None):
                """qT/kT columns [b*512,(b+1)*512) for local head h."""
                ps = pj_pool.tile([128, 512], f32, tag="pj")
                if half is None:
                    c0, c1, w0 = 512 * b, 512 * (b + 1), 512
                else:
                    c0 = 512 * b + 256 * half
                    c1, w0 = c0 + 256, 256
                m0, m1 = 128 * h, 128 * (h + 1)
                chains = ((wh, xh_sb), (wh, xl_sb), (wl, xh_sb))
                for ci, (w, x) in enumerate(chains):
                    for pr in range(KT // 2):
                        nc.tensor.matmul(
                            ps[:, 0:w0],
                            lhsT=w_sb[w][:, 2 * pr:2 * pr + 2, m0:m1],
                            rhs=x[:, 2 * pr:2 * pr + 2, c0:c1],
                            start=(ci == 0 and pr == 0),
                            stop=(ci == 2 and pr == KT // 2 - 1),
                            perf_mode=DR,
                        )
                nc.vector.tensor_copy(dst_sb[:, h, c0:c1], ps[:, 0:w0])

            def proj_v(st):
                """v rows [st*128,(st+1)*128) for both local heads."""
                ps = pj_pool.tile([128, 512], f32, tag="pj")
                r0, r1 = 128 * st, 128 * (st + 1)
                chains = (("wvh", xh_sb), ("wvh", xl_sb), ("wvl", xh_sb))
                for ci, (w, x) in enumerate(chains):
                    for pr in range(KT // 2):
                        nc.tensor.matmul(
                            ps[:, 0:256],
                            lhsT=x[:, 2 * pr:2 * pr + 2, r0:r1],
                            rhs=w_sb[w][:, 2 * pr:2 * pr + 2, :],
                            start=(ci == 0 and pr == 0),
                            stop=(ci == 2 and pr == KT // 2 - 1),
                            perf_mode=DR,
                        )
                nc.vector.tensor_copy(
                    v_sb[:, st, 0:258].rearrange(
                        "p (a c) -> p a c", a=2)[:, :, 0:128],
                    ps[:, 0:256].rearrange("p (a c) -> p a c", a=2))

            def scores(s, h, kt):
                col0 = max(0, (kt - 4 * s) * 128)
                pa = sc_pool.tile([128, 1024], f32, tag="sc")
                nc.tensor.matmul(
                    pa[:, col0:512],
                    lhsT=kT_sb[0:64, h, kt * 128:(kt + 1) * 128],
                    rhs=qT_sb[0:64, h, s * 512 + col0:(s + 1) * 512],
                    start=True, stop=True, tile_position=(0, 0),
                )
                nc.tensor.matmul(
                    pa[:, 512 + col0:1024],
                    lhsT=kT_sb[64:128, h, kt * 128:(kt + 1) * 128],
                    rhs=qT_sb[64:128, h, s * 512 + col0:(s + 1) * 512],
                    start=True, stop=True, tile_position=(64, 0),
                    skip_group_check=True,
                )
                ee = epools[h].tile([128, 1024], f16, tag="e")
                nc.scalar.activation(
                    ee.rearrange("p (b c) -> p b c", b=2)[:, :, col0:512],
                    pa.rearrange("p (b c) -> p b c", b=2)[:, :, col0:512],
                    AF.Exp, scale=SSCALE)
                if kt >= 4 * s:
                    c = col0
                    nc.gpsimd.tensor_mul(ee[:, c:c + 128],
                                         ee[:, c:c + 128], maskt)
                    nc.gpsimd.tensor_mul(ee[:, 512 + c:512 + c + 128],
                                         ee[:, 512 + c:512 + c + 128],
                                         maskt)
                e_tiles[(s, h, kt)] = ee

            def pv(s, h, i):
                """PV chain + per-qtile normalized diff for qtile i of strip s."""
                qt = 4 * s + i
                if (s, h) not in oq_tiles:
                    oq_tiles[(s, h)] = nrm_big.tile(
                        [128, 4, 128], f16, tag="oq_s", name=f"oq_{s}_{h}")
                    ss_tiles[(s, h)] = nrm_big.tile(
                        [128, 4], f32, tag="ss_s", name=f"ss_{s}_{h}")
                oq_s = oq_tiles[(s, h)]
                ss_s = ss_tiles[(s, h)]
                up = up_pool.tile([128, 258], f32, tag="up")
                c = i * 128
                for kt in range(qt + 1):
                    ee = e_tiles[(s, h, kt)]
                    vh = v_sb[:, kt, 129 * h:129 * h + 129]
                    nc.tensor.matmul(
                        up[:, 0:129],
                        lhsT=ee[:, c:c + 128],
                        rhs=vh,
                        start=(kt == 0), stop=(kt == qt),
                    )
                    nc.tensor.matmul(
                        up[:, 129:258],
                        lhsT=ee[:, 512 + c:512 + c + 128],
                        rhs=vh,
                        start=False, stop=(kt == qt),
                        skip_group_check=True,
                    )
                inv0 = nrm.tile([128, 1], f32, tag="inv0")
                nc.vector.reciprocal(inv0, up[:, 128:129])
                inv1 = nrm.tile([128, 1], f32, tag="inv1")
                nc.vector.reciprocal(inv1, up[:, 257:258])
                t1 = nrm.tile([128, 128], f32, tag="t1")
                nc.vector.tensor_scalar(t1, up[:, 129:257], inv1, lam,
                                        OP.mult, OP.mult)
                oq = oq_s[:, i, :]
                nc.vector.scalar_tensor_tensor(
                    oq, up[:, 0:128], inv0, t1, OP.mult, OP.subtract)
                sq = nrm.tile([128, 128], f16, tag="sq")
                nc.vector.scalar_tensor_tensor(
                    sq, oq, 1.0, oq, OP.bypass, OP.mult,
                    accum_out=ss_s[:, i:i + 1])

            def norm_tail(s, h):
                """Newton rsqrt over the strip's 4 q-tiles, scale, transpose."""
                oq_s = oq_tiles[(s, h)]
                ss_s = ss_tiles[(s, h)]
                ms = nrm.tile([128, 4], f32, tag="ms")
                il2 = 1.0 / ((1.0 - LAMBDA_INIT) ** 2)
                nc.vector.tensor_scalar(ms, ss_s, il2 / 128.0,
                                        EPS * il2 * OSCALE2,
                                        OP.mult, OP.add)
                y0 = nrm.tile([128, 4], u32, tag="y0")
                nc.vector.tensor_tensor(y0, ms.bitcast(u32), one_u[:, 0:4],
                                        OP.logical_shift_right)
                nc.vector.tensor_tensor(y0, magic_c[:, 0:4], y0, OP.subtract)
                yf = y0.bitcast(f32)
                t2 = nrm.tile([128, 4], f32, tag="t2")
                r_all = nrm.tile([128, 4], f32, tag="r_all")
                nc.vector.tensor_mul(t2, yf, yf)
                nc.vector.tensor_mul(t2, t2, ms)
                nc.vector.tensor_scalar(t2, t2, -0.5, 1.5, OP.mult, OP.add)
                nc.vector.tensor_mul(r_all, yf, t2)
                nc.vector.tensor_mul(t2, r_all, r_all)
                nc.vector.tensor_mul(t2, t2, ms)
                nc.vector.tensor_scalar(t2, t2, -0.5, 1.5, OP.mult, OP.add)
                nc.vector.tensor_mul(r_all, r_all, t2)
                for i in range(4):
                    qt = 4 * s + i
                    on = osb.tile([128, 128], f16, tag="on")
                    if s == 1:
                        # tail strip: ACT is exp-idle by now; keep DVE clear
                        nc.scalar.activation(on, oq_s[:, i, :], AF.Copy,
                                             scale=r_all[:, i:i + 1])
                    else:
                        nc.vector.tensor_scalar(on, oq_s[:, i, :],
                                                r_all[:, i:i + 1], None,
                                                OP.mult)
                    pt = pj_pool.tile([128, 512], f32, tag="pj")
                    ptv = pt.bitcast(f16)
                    nc.tensor.transpose(ptv[:, 0:128], on, ident)
                    nc.vector.tensor_copy(
                        oT_sb[:, h, qt * 128:(qt + 1) * 128], ptv[:, 0:128])

            ot_tiles = {}

            def outproj(s, mt):
                if s not in ot_tiles:
                    ot_tiles[s] = otp.tile([128, 8, 512], f16, tag="ot",
                                           name=f"ot_{s}")
                ot = ot_tiles[s]
                ps = pj_pool.tile([128, 512], f32, tag="pj")
                for kt2 in range(2):
                    nc.tensor.matmul(
                        ps[:],
                        lhsT=wo_sb[:, kt2, mt * 128:(mt + 1) * 128],
                        rhs=oT_sb[:, kt2, s * 512:(s + 1) * 512],
                        start=(kt2 == 0), stop=(kt2 == 1),
                    )
                nc.vector.tensor_copy(ot[:, mt, :], ps[:])
                if mt == 3:
                    nc.sync.dma_start(
                        outT_r[:, 0:4, s * 512:(s + 1) * 512], ot[:, 0:4, :])
                elif mt == 7:
                    nc.sync.dma_start(
                        outT_r[:, 4:8, s * 512:(s + 1) * 512], ot[:, 4:8, :])

            def outproj_tail(s, mts, pool, width, engines, dma_eng=None):
                """Tail variant: mt group in an idle-ring tile; per-mt evac
                on an explicit engine; one output DMA per group."""
                if s not in ot_tiles:
                    ot_tiles[s] = otp.tile([128, 8, 512], f16, tag="ot",
                                           name=f"ot_{s}")
                ot = ot_tiles[s]
                tag = "sc" if width == 1024 else "pj"
                ps = pool.tile([128, width], f32, tag=tag)
                for mi, mt in enumerate(mts):
                    for kt2 in range(2):
                        nc.tensor.matmul(
                            ps[:, mi * 512:(mi + 1) * 512],
                            lhsT=wo_sb[:, kt2, mt * 128:(mt + 1) * 128],
                            rhs=oT_sb[:, kt2, s * 512:(s + 1) * 512],
                            start=(kt2 == 0), stop=(kt2 == 1),
                            skip_group_check=(mi == 1),
                        )
                for mi, mt in enumerate(mts):
                    sl = ps[:, mi * 512:(mi + 1) * 512]
                    if engines[mi] == "dve":
                        nc.vector.tensor_copy(ot[:, mt, :], sl)
                    else:
                        nc.scalar.copy(ot[:, mt, :], sl)
                    (dma_eng or nc.sync).dma_start(
                        outT_r[:, mt, s * 512:(s + 1) * 512], ot[:, mt, :])

            # ---------------- flat schedule ----------------
            U = []  # list of thunks

            def k_u(h, b, half=None):
                return lambda: proj_qk("wkh", "wkl", kT_sb, h, b, half)

            def q_u(h, b, half=None):
                return lambda: proj_qk("wqh", "wql", qT_sb, h, b, half)

            def v_u(st):
                return lambda: proj_v(st)

            def sc_u(s, h, kt):
                return lambda: scores(s, h, kt)

            def pv_u(s, h, i):
                return lambda: pv(s, h, i)

            def nt_u(s, h):
                return lambda: norm_tail(s, h)

            def op_u(s, mt):
                return lambda: outproj(s, mt)

            def a_phase(s, fillers, extra=None):
                fi = iter(fillers)
                for kt in range(4 * s + 4):
                    U.append(sc_u(s, 0, kt))
                    U.append(sc_u(s, 1, kt))
                    for _ in range(2):
                        f = next(fi, None)
                        if f is not None:
                            U.append(f)
                    if extra and kt in extra:
                        U.extend(extra[kt])
                rest = list(fi)
                U.extend(rest)

            U += [k_u(0, 0), k_u(1, 0), q_u(0, 0), q_u(1, 0)]

            a_phase(0, [v_u(0), v_u(1), v_u(2), v_u(3),
                        k_u(0, 1), k_u(1, 1), q_u(0, 2), q_u(1, 2)])

            a_phase(2, [pv_u(0, 0, 0), pv_u(0, 1, 0),
                        pv_u(0, 0, 1), pv_u(0, 1, 1),
                        k_u(0, 2), k_u(1, 2),
                        pv_u(0, 0, 2), pv_u(0, 1, 2),
                        pv_u(0, 0, 3), pv_u(0, 1, 3),
                        nt_u(0, 0), nt_u(0, 1),
                        q_u(0, 3), q_u(1, 3),
                        v_u(4), v_u(5), v_u(6), v_u(7), v_u(8), v_u(9),
                        v_u(10), v_u(11),
                        op_u(0, 0), op_u(0, 1)])

            a_phase(3, [k_u(0, 3), k_u(1, 3),
                        op_u(0, 2), op_u(0, 3),
                        pv_u(2, 0, 0), pv_u(2, 1, 0),
                        pv_u(2, 0, 1), pv_u(2, 1, 1),
                        op_u(0, 4), op_u(0, 5),
                        pv_u(2, 0, 2), pv_u(2, 1, 2),
                        pv_u(2, 0, 3), pv_u(2, 1, 3),
                        nt_u(2, 0), nt_u(2, 1),
                        op_u(0, 6), op_u(0, 7),
                        op_u(2, 0), op_u(2, 1), op_u(2, 2), op_u(2, 3),
                        op_u(2, 4), op_u(2, 5), op_u(2, 6), op_u(2, 7),
                        q_u(0, 1), q_u(1, 1),
                        v_u(12), v_u(13), v_u(14), v_u(15),
                        pv_u(3, 0, 0), pv_u(3, 1, 0)])

            a_phase(1, [pv_u(3, 0, 1), pv_u(3, 1, 1),
                        pv_u(3, 0, 2), pv_u(3, 1, 2),
                        pv_u(3, 0, 3), pv_u(3, 1, 3),
                        nt_u(3, 0), nt_u(3, 1),
                        op_u(3, 0), op_u(3, 1), op_u(3, 2), op_u(3, 3),
                        op_u(3, 4), op_u(3, 5), op_u(3, 6), op_u(3, 7)],
                    extra={5: [pv_u(1, 0, 0)],
                           6: [pv_u(1, 0, 1)],
                           7: [pv_u(1, 0, 2)]})

            U += [pv_u(1, 0, 3),
                  nt_u(1, 0),
                  pv_u(1, 1, 0), pv_u(1, 1, 1),
                  pv_u(1, 1, 2), pv_u(1, 1, 3),
                  nt_u(1, 1),
                  (lambda: outproj_tail(1, (0, 1), sc_pool, 1024,
                                        ("dve", "act"))),
                  (lambda: outproj_tail(1, (2, 3), sc_pool, 1024,
                                        ("act", "dve"), nc.gpsimd)),
                  (lambda: outproj_tail(1, (4,), pj_pool, 512, ("dve",))),
                  (lambda: outproj_tail(1, (5,), pj_pool, 512, ("act",),
                                        nc.gpsimd)),
                  (lambda: outproj_tail(1, (6, 7), sc_pool, 1024,
                                        ("dve", "act")))]

            for u in U:
                u()

    nc.compile()
    return nc


def _split_fp8(a):
    import ml_dtypes
    f8 = ml_dtypes.float8_e4m3
    hi = a.astype(f8)
    lo = (a - hi.astype(np.float32)).astype(f8)
    return hi, lo


def _interleave_w(wT):
    """[D, 256] -> [128, KT*256] with arr[p, kt*256+m] = wT[kt*128+p, m]."""
    return np.ascontiguousarray(
        wT.reshape(KT, 128, 256).transpose(1, 0, 2).reshape(128, KT * 256))


def _prep_inputs(x, Wq, Wk, Wv, Wo):
    """Build the 8 per-core input maps (host-side shard/split/transpose)."""
    f16 = np.float16
    scale = HD ** -0.5
    xs = []
    for b in range(B):
        xT = np.ascontiguousarray(x[b].T).astype(np.float32)
        xs.append(_split_fp8(xT))
    in_maps = []
    for d in range(N_CORES):
        b, p = divmod(d, 4)
        r0 = 256 * p
        xh, xl = xs[b]
        wq = np.ascontiguousarray(Wq[r0:r0 + 256, :].T) * (scale * QSCALE)
        wk = np.ascontiguousarray(Wk[r0:r0 + 256, :].T) * KSCALE
        wv = np.ascontiguousarray(Wv[r0:r0 + 256, :].T) * VSCALE
        wqh, wql = _split_fp8(wq.astype(np.float32))
        wkh, wkl = _split_fp8(wk.astype(np.float32))
        wvh, wvl = _split_fp8(wv.astype(np.float32))
        in_maps.append({
            "xh": xh, "xl": xl,
            "wqh": _interleave_w(wqh), "wql": _interleave_w(wql),
            "wkh": _interleave_w(wkh), "wkl": _interleave_w(wkl),
            "wvh": _interleave_w(wvh), "wvl": _interleave_w(wvl),
            "woT": np.ascontiguousarray(Wo[:, r0:r0 + 256].T).astype(f16),
        })
    return in_maps


_CACHED = {}


def _get_program(lam: float):
    # the program depends on inputs only through lam
    key = round(float(lam), 9)
    if key not in _CACHED:
        _CACHED[key] = _build_program(float(lam))
    return _CACHED[key]


def kernel(x, Wq, Wk, Wv, Wo, lq1, lk1, lq2, lk2):
    from concourse.bass_utils import run_bass_kernel_spmd

    x = np.asarray(x, dtype=np.float32)
    Wq = np.asarray(Wq, dtype=np.float32)
    Wk = np.asarray(Wk, dtype=np.float32)
    Wv = np.asarray(Wv, dtype=np.float32)
    Wo = np.asarray(Wo, dtype=np.float32)
    lq1 = np.asarray(lq1, dtype=np.float32)
    lk1 = np.asarray(lk1, dtype=np.float32)
    lq2 = np.asarray(lq2, dtype=np.float32)
    lk2 = np.asarray(lk2, dtype=np.float32)

    lam1 = np.exp(np.sum(lq1 * lk1, dtype=np.float32))
    lam2 = np.exp(np.sum(lq2 * lk2, dtype=np.float32))
    lam = float(lam1 - lam2 + LAMBDA_INIT)

    nc = _get_program(lam)
    in_maps = _prep_inputs(x, Wq, Wk, Wv, Wo)
    res = run_bass_kernel_spmd(nc, in_maps, core_ids=list(range(N_CORES)))

    out = np.empty((B, S, D), dtype=np.float32)
    for b in range(B):
        acc = res.results[4 * b]["outT"].astype(np.float32)
        for p in range(1, 4):
            acc += res.results[4 * b + p]["outT"].astype(np.float32)
        out[b] = acc.T
    return out


# revision 76
# speedup vs baseline: 1.2744x; 1.0326x over previous
"""DiffAttention Trainium2 kernel (fp8 hi/lo projections + pipelined schedule).

Full inputs in, full output out. Sharding: 8 cores = (batch b in {0,1}) x
(head-pair p in {0..3}); each core handles one batch element and 2 of the 8
heads (= 4 of the 16 q/k half-heads, 2 v heads, 256 of the 1024 o columns).
Out-projection is column-split: each core produces a full (S, D) partial of
o @ Wo.T restricted to its o columns; host sums the 4 partials per batch.

Projections run as fp8e4m3 DoubleRow matmuls with host-side error
compensation: x and each W shard are split hi/lo (hi = fp8(t), lo =
fp8(t - hi)) and the three significant products xh@Wh + xh@Wl + xl@Wh are
accumulated in PSUM (the dropped xl@Wl term is ~0.07% relative).  DoubleRow
contracts two 128-deep k-tiles per instruction at 0.5 cycles/row, so each
projection costs 6 rows/out-tile instead of fp16's 8.  Weights carry
power-of-2 pre-scales (q: 2^7*hd^-0.5, k/v: 2^4) to center fp8 exponents;
the combined 2^-11 is folded into the exp's scale argument and the RMSNorm
epsilon (the norm itself is scale-invariant), so no evacuation rescale is
needed anywhere.

Attention math per head h (half-heads e0=2h, e1=2h+1), per q row:
  u_i = exp(s_i) @ v   (unnormalized), sum_i = exp(s_i) @ 1  (fused: rhs=[v|1])
  o   = u0/sum0 - lam * u1/sum1
  o   = o * rsqrt(mean(o^2)+eps) * (1-lam_init);   out = o @ Wo.T
Scores are computed transposed (keys on partitions, q on free dim) so the
exp'd tiles feed the PV matmul directly as the stationary operand.  rsqrt is
Newton-Raphson on the DVE (fast-inverse-sqrt seed), batched per (strip, head).

Scheduling: a single flat unit stream, strip order [0,2,3,1] (small strip
last to shrink the drain tail).  Each strip's score+exp units are
interleaved with filler PE work (previous strip's PV chains / norm tails /
out-projection chunks, projection units, v-projection tiles) so the PE never
waits on the ACT exp pipeline and never idles long enough to drop out of its
high p-state.  Input DMA is chunked (x in 512-column blocks — 512-byte
per-partition runs, the minimum for full DMA descriptor rate; weights
hi-before-lo) and ordered by first use, with xh on the SP trigger queue and
xl on the ACT queue so the two streams transfer concurrently.  Evacuations
are balanced across DVE/ACT (GPSIMD cannot read PSUM); the tail strip's
out-projection spreads over four PSUM slots (two idle scores-ring
[128,1024] pairs + two [128,512] singles) with evacs split DVE/ACT in
parallel and per-mt output DMAs on alternating SP/GPSIMD queues, shortening
the final matmul->evac->DMA drain.  PSUM: scores 2x[128,1024] + PV
2x[128,258] + a shared [128,512] ring (projection evac / out-projection /
transposes, which stay at 1 cycle/row because the identity operand is f16)
= exactly 8 banks.
"""

import math

import numpy as np

B = 2
S = 2048
D = 1024
H = 8
HD = 64  # half-head dim
LAMBDA_INIT = 0.8 - 0.6 * math.exp(-0.3 * 6)
EPS = 1e-5

N_CORES = 8
KT = D // 128      # 8 contraction tiles for projections
ST = S // 128      # 16 sequence tiles
NSTRIP = S // 512  # 4 q strips

QSCALE = 2.0 ** 7   # folded into WqT (on top of hd^-0.5)
KSCALE = 2.0 ** 4   # folded into WkT
VSCALE = 2.0 ** 4   # folded into WvT
SSCALE = 1.0 / (QSCALE * KSCALE)   # exp() input scale
OSCALE2 = float(VSCALE * VSCALE)   # o is VSCALE-scaled; ss is VSCALE^2-scaled


def _build_program(lam: float):
    import concourse.bass as bass
    import concourse.tile as tile
    from concourse import bacc, mybir
    from concourse.masks import make_identity

    f8 = mybir.dt.float8e4
    f16 = mybir.dt.float16
    f32 = mybir.dt.float32
    u32 = mybir.dt.uint32
    AF = mybir.ActivationFunctionType
    OP = mybir.AluOpType
    DR = mybir.MatmulPerfMode.DoubleRow

    nc = bacc.Bacc("TRN2", target_bir_lowering=False, debug=False,
                   num_devices=N_CORES)

    xh_d = nc.dram_tensor("xh", (D, S), f8, kind="ExternalInput").ap()
    xl_d = nc.dram_tensor("xl", (D, S), f8, kind="ExternalInput").ap()
    # weights host-interleaved to [128, KT*256] for contiguous 2KB DMA runs
    w_d = {}
    for nm in ("wqh", "wql", "wkh", "wkl", "wvh", "wvl"):
        w_d[nm] = nc.dram_tensor(nm, (128, KT * 256), f8,
                                 kind="ExternalInput").ap()
    woT = nc.dram_tensor("woT", (256, D), f16, kind="ExternalInput").ap()
    outT = nc.dram_tensor("outT", (D, S), f16, kind="ExternalOutput").ap()

    with tile.TileContext(nc) as tc:
        with (
            tc.tile_pool(name="const", bufs=1) as cpool,
            tc.tile_pool(name="persist", bufs=1) as pp,
            tc.tile_pool(name="pin", bufs=1) as pin,
            tc.tile_pool(name="e0p", bufs=24) as e0pool,
            tc.tile_pool(name="e1p", bufs=24) as e1pool,
            tc.tile_pool(name="sc", bufs=2, space="PSUM") as sc_pool,
            tc.tile_pool(name="up", bufs=2, space="PSUM") as up_pool,
            tc.tile_pool(name="pj", bufs=2, space="PSUM") as pj_pool,
            tc.tile_pool(name="nrm", bufs=8) as nrm,
            tc.tile_pool(name="nrm_big", bufs=2) as nrm_big,
            tc.tile_pool(name="osb", bufs=3) as osb,
            tc.tile_pool(name="otp", bufs=2) as otp,
        ):
            epools = {0: e0pool, 1: e1pool}

            ident = cpool.tile([128, 128], f16, tag="ident")
            make_identity(nc, ident)
            # mask[p, f] = 1 if p <= f else 0 (keys on partitions, q on free)
            maskt = cpool.tile([128, 128], f16, tag="maskt")
            nc.gpsimd.memset(maskt, 1.0)
            nc.gpsimd.affine_select(
                out=maskt, in_=maskt, compare_op=OP.is_ge, fill=0.0,
                base=0, pattern=[[1, 128]], channel_multiplier=-1,
            )
            # constants for Newton-Raphson rsqrt (fast-inverse-sqrt seed)
            magic_c = cpool.tile([128, 8], u32, tag="magic_c")
            nc.gpsimd.memset(magic_c, 0x5F3759DF)
            one_u = cpool.tile([128, 8], u32, tag="one_u")
            nc.gpsimd.memset(one_u, 1)

            qT_sb = pp.tile([128, 2, S], f16, tag="qT_sb")
            kT_sb = pp.tile([128, 2, S], f16, tag="kT_sb")
            v_sb = pp.tile([128, ST, 258], f16, tag="v_sb")
            nc.vector.memset(v_sb[:, :, 128:129], 1.0)
            nc.vector.memset(v_sb[:, :, 257:258], 1.0)
            oT_sb = pp.tile([128, 2, S], f16, tag="oT_sb")
            wo_sb = pp.tile([128, 2, D], f16, tag="wo_sb")

            xh_sb = pin.tile([128, KT, S], f8, tag="xh_sb")
            xl_sb = pin.tile([128, KT, S], f8, tag="xl_sb")
            w_sb = {}
            for nm in ("wqh", "wql", "wkh", "wkl", "wvh", "wvl"):
                wt = pin.tile([128, KT, 256], f8, tag=nm + "_sb",
                              name=nm + "_sb")
                w_sb[nm] = wt

            # ---------------- input DMAs, ordered by first use -------------
            xh_r = xh_d.rearrange("(kt p) s -> p kt s", p=128)
            xl_r = xl_d.rearrange("(kt p) s -> p kt s", p=128)

            def dma_w(nm, eng=None):
                (eng or nc.sync).dma_start(
                    w_sb[nm][:, :, :],
                    w_d[nm].rearrange("p (kt m) -> p kt m", m=256)[:, :, :])

            def dma_x(b):
                c0, c1 = 512 * b, 512 * (b + 1)
                nc.sync.dma_start(xh_sb[:, :, c0:c1], xh_r[:, :, c0:c1])
                nc.gpsimd.dma_start(xl_sb[:, :, c0:c1], xl_r[:, :, c0:c1])

            def dma_x_half(b, half):
                c0 = 512 * b + 256 * half
                c1 = c0 + 256
                nc.sync.dma_start(xh_sb[:, :, c0:c1], xh_r[:, :, c0:c1])
                nc.sync.dma_start(xl_sb[:, :, c0:c1], xl_r[:, :, c0:c1])

            dma_w("wkh")
            dma_x(0)
            dma_w("wkl")
            dma_w("wqh")
            dma_w("wql", nc.gpsimd)
            dma_w("wvh", nc.gpsimd)
            dma_w("wvl", nc.gpsimd)
            dma_x(1)
            dma_x(2)
            nc.gpsimd.dma_start(
                wo_sb[:, :, :],
                woT.rearrange("(kt p) n -> p kt n", p=128)[:, :, :])
            dma_x(3)

            # ---------------- unit closures ----------------
            e_tiles = {}     # (s, h, kt) -> SBUF tile [128, 1024] f16
            oq_tiles = {}    # (s, h) -> [128, 4, 128] f32
            ss_tiles = {}    # (s, h) -> [128, 4] f32
            outT_r = outT.rearrange("(mt p) s -> p mt s", p=128)

            def proj_qk(wh, wl, dst_sb, h, b, half=None):
                """qT/kT columns [b*512,(b+1)*512) for local head h."""
                ps = pj_pool.tile([128, 512], f32, tag="pj")
                if half is None:
                    c0, c1, w0 = 512 * b, 512 * (b + 1), 512
                else:
                    c0 = 512 * b + 256 * half
                    c1, w0 = c0 + 256, 256
                m0, m1 = 128 * h, 128 * (h + 1)
                chains = ((wh, xh_sb), (wh, xl_sb), (wl, xh_sb))
                for ci, (w, x) in enumerate(chains):
                    for pr in range(KT // 2):
                        nc.tensor.matmul(
                            ps[:, 0:w0],
                            lhsT=w_sb[w][:, 2 * pr:2 * pr + 2, m0:m1],
                            rhs=x[:, 2 * pr:2 * pr + 2, c0:c1],
                            start=(ci == 0 and pr == 0),
                            stop=(ci == 2 and pr == KT // 2 - 1),
                            perf_mode=DR,
                        )
                nc.vector.tensor_copy(dst_sb[:, h, c0:c1], ps[:, 0:w0])

            def proj_v(st):
                """v rows [st*128,(st+1)*128) for both local heads."""
                ps = pj_pool.tile([128, 512], f32, tag="pj")
                r0, r1 = 128 * st, 128 * (st + 1)
                chains = (("wvh", xh_sb), ("wvh", xl_sb), ("wvl", xh_sb))
                for ci, (w, x) in enumerate(chains):
                    for pr in range(KT // 2):
                        nc.tensor.matmul(
                            ps[:, 0:256],
                            lhsT=x[:, 2 * pr:2 * pr + 2, r0:r1],
                            rhs=w_sb[w][:, 2 * pr:2 * pr + 2, :],
                            start=(ci == 0 and pr == 0),
                            stop=(ci == 2 and pr == KT // 2 - 1),
                            perf_mode=DR,
                        )
                nc.vector.tensor_copy(
                    v_sb[:, st, 0:258].rearrange(
                        "p (a c) -> p a c", a=2)[:, :, 0:128],
                    ps[:, 0:256].rearrange("p (a c) -> p a c", a=2))

            def scores(s, h, kt):
                col0 = max(0, (kt - 4 * s) * 128)
                pa = sc_pool.tile([128, 1024], f32, tag="sc")
                nc.tensor.matmul(
                    pa[:, col0:512],
                    lhsT=kT_sb[0:64, h, kt * 128:(kt + 1) * 128],
                    rhs=qT_sb[0:64, h, s * 512 + col0:(s + 1) * 512],
                    start=True, stop=True, tile_position=(0, 0),
                )
                nc.tensor.matmul(
                    pa[:, 512 + col0:1024],
                    lhsT=kT_sb[64:128, h, kt * 128:(kt + 1) * 128],
                    rhs=qT_sb[64:128, h, s * 512 + col0:(s + 1) * 512],
                    start=True, stop=True, tile_position=(64, 0),
                    skip_group_check=True,
                )
                ee = epools[h].tile([128, 1024], f16, tag="e")
                nc.scalar.activation(
                    ee.rearrange("p (b c) -> p b c", b=2)[:, :, col0:512],
                    pa.rearrange("p (b c) -> p b c", b=2)[:, :, col0:512],
                    AF.Exp, scale=SSCALE)
                if kt >= 4 * s:
                    c = col0
                    nc.gpsimd.tensor_mul(ee[:, c:c + 128],
                                         ee[:, c:c + 128], maskt)
                    nc.gpsimd.tensor_mul(ee[:, 512 + c:512 + c + 128],
                                         ee[:, 512 + c:512 + c + 128],
                                         maskt)
                e_tiles[(s, h, kt)] = ee

            def pv(s, h, i):
                """PV chain + per-qtile normalized diff for qtile i of strip s."""
                qt = 4 * s + i
                if (s, h) not in oq_tiles:
                    oq_tiles[(s, h)] = nrm_big.tile(
                        [128, 4, 128], f16, tag="oq_s", name=f"oq_{s}_{h}")
                    ss_tiles[(s, h)] = nrm_big.tile(
                        [128, 4], f32, tag="ss_s", name=f"ss_{s}_{h}")
                oq_s = oq_tiles[(s, h)]
                ss_s = ss_tiles[(s, h)]
                up = up_pool.tile([128, 258], f32, tag="up")
                c = i * 128
                for kt in range(qt + 1):
                    ee = e_tiles[(s, h, kt)]
                    vh = v_sb[:, kt, 129 * h:129 * h + 129]
                    nc.tensor.matmul(
                        up[:, 0:129],
                        lhsT=ee[:, c:c + 128],
                        rhs=vh,
                        start=(kt == 0), stop=(kt == qt),
                    )
                    nc.tensor.matmul(
                        up[:, 129:258],
                        lhsT=ee[:, 512 + c:512 + c + 128],
                        rhs=vh,
                        start=False, stop=(kt == qt),
                        skip_group_check=True,
                    )
                inv0 = nrm.tile([128, 1], f32, tag="inv0")
                nc.vector.reciprocal(inv0, up[:, 128:129])
                inv1 = nrm.tile([128, 1], f32, tag="inv1")
                nc.vector.reciprocal(inv1, up[:, 257:258])
                t1 = nrm.tile([128, 128], f32, tag="t1")
                nc.vector.tensor_scalar(t1, up[:, 129:257], inv1, lam,
                                        OP.mult, OP.mult)
                oq = oq_s[:, i, :]
                nc.vector.scalar_tensor_tensor(
                    oq, up[:, 0:128], inv0, t1, OP.mult, OP.subtract)
                sq = nrm.tile([128, 128], f16, tag="sq")
                nc.vector.scalar_tensor_tensor(
                    sq, oq, 1.0, oq, OP.bypass, OP.mult,
                    accum_out=ss_s[:, i:i + 1])

            def norm_tail(s, h):
                """Newton rsqrt over the strip's 4 q-tiles, scale, transpose."""
                oq_s = oq_tiles[(s, h)]
                ss_s = ss_tiles[(s, h)]
                ms = nrm.tile([128, 4], f32, tag="ms")
                il2 = 1.0 / ((1.0 - LAMBDA_INIT) ** 2)
                nc.vector.tensor_scalar(ms, ss_s, il2 / 128.0,
                                        EPS * il2 * OSCALE2,
                                        OP.mult, OP.add)
                y0 = nrm.tile([128, 4], u32, tag="y0")
                nc.vector.tensor_tensor(y0, ms.bitcast(u32), one_u[:, 0:4],
                                        OP.logical_shift_right)
                nc.vector.tensor_tensor(y0, magic_c[:, 0:4], y0, OP.subtract)
                yf = y0.bitcast(f32)
                t2 = nrm.tile([128, 4], f32, tag="t2")
                r_all = nrm.tile([128, 4], f32, tag="r_all")
                nc.vector.tensor_mul(t2, yf, yf)
                nc.vector.tensor_mul(t2, t2, ms)
                nc.vector.tensor_scalar(t2, t2, -0.5, 1.5, OP.mult, OP.add)
                nc.vector.tensor_mul(r_all, yf, t2)
                nc.vector.tensor_mul(t2, r_all, r_all)
                nc.vector.tensor_mul(t2, t2, ms)
                nc.vector.tensor_scalar(t2, t2, -0.5, 1.5, OP.mult, OP.add)
                nc.vector.tensor_mul(r_all, r_all, t2)
                for i in range(4):
                    qt = 4 * s + i
                    on = osb.tile([128, 128], f16, tag="on")
                    if s == 1:
                        # tail strip: ACT is exp-idle by now; keep DVE clear
                        nc.scalar.activation(on, oq_s[:, i, :], AF.Copy,
                                             scale=r_all[:, i:i + 1])
                    else:
                        nc.vector.tensor_scalar(on, oq_s[:, i, :],
                                                r_all[:, i:i + 1], None,
                                                OP.mult)
                    pt = pj_pool.tile([128, 512], f32, tag="pj")
                    ptv = pt.bitcast(f16)
                    nc.tensor.transpose(ptv[:, 0:128], on, ident)
                    nc.vector.tensor_copy(
                        oT_sb[:, h, qt * 128:(qt + 1) * 128], ptv[:, 0:128])

            ot_tiles = {}

            def outproj(s, mt):
                if s not in ot_tiles:
                    ot_tiles[s] = otp.tile([128, 8, 512], f16, tag="ot",
                                           name=f"ot_{s}")
                ot = ot_tiles[s]
                ps = pj_pool.tile([128, 512], f32, tag="pj")
                for kt2 in range(2):
                    nc.tensor.matmul(
                        ps[:],
                        lhsT=wo_sb[:, kt2, mt * 128:(mt + 1) * 128],
                        rhs=oT_sb[:, kt2, s * 512:(s + 1) * 512],
                        start=(kt2 == 0), stop=(kt2 == 1),
                    )
                if mt % 2 == 1:
                    nc.scalar.copy(ot[:, mt, :], ps[:])
                else:
                    nc.vector.tensor_copy(ot[:, mt, :], ps[:])
                if mt == 3:
                    nc.sync.dma_start(
                        outT_r[:, 0:4, s * 512:(s + 1) * 512], ot[:, 0:4, :])
                elif mt == 7:
                    nc.sync.dma_start(
                        outT_r[:, 4:8, s * 512:(s + 1) * 512], ot[:, 4:8, :])

            def outproj_tail(s, mts, pool, width, engines, dma_eng=None):
                """Tail variant: mt group in an idle-ring tile; per-mt evac
                on an explicit engine; one output DMA per group."""
                if s not in ot_tiles:
                    ot_tiles[s] = otp.tile([128, 8, 512], f16, tag="ot",
                                           name=f"ot_{s}")
                ot = ot_tiles[s]
                tag = "sc" if width == 1024 else "pj"
                ps = pool.tile([128, width], f32, tag=tag)
                for mi, mt in enumerate(mts):
                    for kt2 in range(2):
                        nc.tensor.matmul(
                            ps[:, mi * 512:(mi + 1) * 512],
                            lhsT=wo_sb[:, kt2, mt * 128:(mt + 1) * 128],
                            rhs=oT_sb[:, kt2, s * 512:(s + 1) * 512],
                            start=(kt2 == 0), stop=(kt2 == 1),
                            skip_group_check=(mi == 1),
                        )
                for mi, mt in enumerate(mts):
                    sl = ps[:, mi * 512:(mi + 1) * 512]
                    if engines[mi] == "dve":
                        nc.vector.tensor_copy(ot[:, mt, :], sl)
                    else:
                        nc.scalar.copy(ot[:, mt, :], sl)
                    (dma_eng or nc.sync).dma_start(
                        outT_r[:, mt, s * 512:(s + 1) * 512], ot[:, mt, :])

            # ---------------- flat schedule ----------------
            U = []  # list of thunks

            def k_u(h, b, half=None):
                return lambda: proj_qk("wkh", "wkl", kT_sb, h, b, half)

            def q_u(h, b, half=None):
                return lambda: proj_qk("wqh", "wql", qT_sb, h, b, half)

            def v_u(st):
                return lambda: proj_v(st)

            def sc_u(s, h, kt):
                return lambda: scores(s, h, kt)

            def pv_u(s, h, i):
                return lambda: pv(s, h, i)

            def nt_u(s, h):
                return lambda: norm_tail(s, h)

            def op_u(s, mt):
                return lambda: outproj(s, mt)

            def a_phase(s, fillers, extra=None):
                fi = iter(fillers)
                for kt in range(4 * s + 4):
                    U.append(sc_u(s, 0, kt))
                    U.append(sc_u(s, 1, kt))
                    for _ in range(2):
                        f = next(fi, None)
                        if f is not None:
                            U.append(f)
                    if extra and kt in extra:
                        U.extend(extra[kt])
                rest = list(fi)
                U.extend(rest)

            U += [k_u(0, 0), k_u(1, 0), q_u(0, 0), q_u(1, 0)]

            a_phase(0, [v_u(0), v_u(1), v_u(2), v_u(3),
                        k_u(0, 1), k_u(1, 1), q_u(0, 2), q_u(1, 2)])

            a_phase(2, [pv_u(0, 0, 0), k_u(0, 2),
                        pv_u(0, 1, 0), k_u(1, 2),
                        pv_u(0, 0, 1), q_u(0, 3),
                        pv_u(0, 1, 1), q_u(1, 3),
                        pv_u(0, 0, 2), v_u(4),
                        pv_u(0, 1, 2), v_u(5),
                        pv_u(0, 0, 3), v_u(6),
                        pv_u(0, 1, 3), v_u(7),
                        nt_u(0, 0), nt_u(0, 1),
                        v_u(8), v_u(9), v_u(10), v_u(11),
                        op_u(0, 0), op_u(0, 1)])

            a_phase(3, [k_u(0, 3), k_u(1, 3),
                        pv_u(2, 0, 0), op_u(0, 2),
                        pv_u(2, 1, 0), op_u(0, 3),
                        pv_u(2, 0, 1), op_u(0, 4),
                        pv_u(2, 1, 1), op_u(0, 5),
                        pv_u(2, 0, 2), op_u(0, 6),
                        pv_u(2, 1, 2), op_u(0, 7),
                        pv_u(2, 0, 3), q_u(0, 1),
                        pv_u(2, 1, 3), q_u(1, 1),
                        nt_u(2, 0), nt_u(2, 1),
                        op_u(2, 0), v_u(12), op_u(2, 1), v_u(13),
                        op_u(2, 2), v_u(14), op_u(2, 3), v_u(15),
                        op_u(2, 4), op_u(2, 5), op_u(2, 6), op_u(2, 7),
                        pv_u(3, 0, 0), pv_u(3, 1, 0)])

            a_phase(1, [pv_u(3, 0, 1), pv_u(3, 1, 1),
                        pv_u(3, 0, 2), pv_u(3, 1, 2),
                        pv_u(3, 0, 3), pv_u(3, 1, 3),
                        nt_u(3, 0), nt_u(3, 1),
                        op_u(3, 0), pv_u(1, 0, 0),
                        op_u(3, 1), pv_u(1, 0, 1),
                        op_u(3, 2), pv_u(1, 0, 2),
                        op_u(3, 3), op_u(3, 4)])

            tail_ps = {}

            def opt_k0(key, mts):
                """First (h0) contraction half of a tail mt-pair: runs as
                soon as nt(1,0) lands, overlapping the h1 PV chains."""
                ps = sc_pool.tile([128, 1024], f32, tag="sc")
                tail_ps[key] = ps
                for mi, mt in enumerate(mts):
                    nc.tensor.matmul(
                        ps[:, mi * 512:(mi + 1) * 512],
                        lhsT=wo_sb[:, 0, mt * 128:(mt + 1) * 128],
                        rhs=oT_sb[:, 0, 512:1024],
                        start=True, stop=False,
                        skip_group_check=True,
                    )

            def opt_k1(key, mts, engines, dma_eng=None):
                ps = tail_ps[key]
                if 1 not in ot_tiles:
                    ot_tiles[1] = otp.tile([128, 8, 512], f16, tag="ot",
                                           name="ot_1")
                ot = ot_tiles[1]
                for mi, mt in enumerate(mts):
                    nc.tensor.matmul(
                        ps[:, mi * 512:(mi + 1) * 512],
                        lhsT=wo_sb[:, 1, mt * 128:(mt + 1) * 128],
                        rhs=oT_sb[:, 1, 512:1024],
                        start=False, stop=True,
                        skip_group_check=True,
                    )
                for mi, mt in enumerate(mts):
                    sl = ps[:, mi * 512:(mi + 1) * 512]
                    if engines[mi] == "dve":
                        nc.vector.tensor_copy(ot[:, mt, :], sl)
                    else:
                        nc.scalar.copy(ot[:, mt, :], sl)
                    (dma_eng or nc.sync).dma_start(
                        outT_r[:, mt, 512:1024], ot[:, mt, :])

            U += [op_u(3, 5), pv_u(1, 0, 3),
                  op_u(3, 6), nt_u(1, 0),
                  (lambda: opt_k0("A", (0, 1))),
                  op_u(3, 7),
                  pv_u(1, 1, 0),
                  (lambda: opt_k0("B", (2, 3))),
                  pv_u(1, 1, 1),
                  pv_u(1, 1, 2), pv_u(1, 1, 3),
                  nt_u(1, 1),
                  (lambda: opt_k1("A", (0, 1), ("dve", "act"))),
                  (lambda: opt_k1("B", (2, 3), ("act", "dve"), nc.gpsimd)),
                  (lambda: outproj_tail(1, (4,), pj_pool, 512, ("dve",))),
                  (lambda: outproj_tail(1, (5,), pj_pool, 512, ("act",),
                                        nc.gpsimd)),
                  (lambda: outproj_tail(1, (6, 7), sc_pool, 1024,
                                        ("dve", "act")))]

            for u in U:
                u()

    nc.compile()
    return nc


def _split_fp8(a):
    import ml_dtypes
    f8 = ml_dtypes.float8_e4m3
    hi = a.astype(f8)
    lo = (a - hi.astype(np.float32)).astype(f8)
    return hi, lo


def _interleave_w(wT):
    """[D, 256] -> [128, KT*256] with arr[p, kt*256+m] = wT[kt*128+p, m]."""
    return np.ascontiguousarray(
        wT.reshape(KT, 128, 256).transpose(1, 0, 2).reshape(128, KT * 256))


def _prep_inputs(x, Wq, Wk, Wv, Wo):
    """Build the 8 per-core input maps (host-side shard/split/transpose)."""
    f16 = np.float16
    scale = HD ** -0.5
    xs = []
    for b in range(B):
        xT = np.ascontiguousarray(x[b].T).astype(np.float32)
        xs.append(_split_fp8(xT))
    in_maps = []
    for d in range(N_CORES):
        b, p = divmod(d, 4)
        r0 = 256 * p
        xh, xl = xs[b]
        wq = np.ascontiguousarray(Wq[r0:r0 + 256, :].T) * (scale * QSCALE)
        wk = np.ascontiguousarray(Wk[r0:r0 + 256, :].T) * KSCALE
        wv = np.ascontiguousarray(Wv[r0:r0 + 256, :].T) * VSCALE
        wqh, wql = _split_fp8(wq.astype(np.float32))
        wkh, wkl = _split_fp8(wk.astype(np.float32))
        wvh, wvl = _split_fp8(wv.astype(np.float32))
        in_maps.append({
            "xh": xh, "xl": xl,
            "wqh": _interleave_w(wqh), "wql": _interleave_w(wql),
            "wkh": _interleave_w(wkh), "wkl": _interleave_w(wkl),
            "wvh": _interleave_w(wvh), "wvl": _interleave_w(wvl),
            "woT": np.ascontiguousarray(Wo[:, r0:r0 + 256].T).astype(f16),
        })
    return in_maps


_CACHED = {}


def _get_program(lam: float):
    # the program depends on inputs only through lam
    key = round(float(lam), 9)
    if key not in _CACHED:
        _CACHED[key] = _build_program(float(lam))
    return _CACHED[key]


def kernel(x, Wq, Wk, Wv, Wo, lq1, lk1, lq2, lk2):
    from concourse.bass_utils import run_bass_kernel_spmd

    x = np.asarray(x, dtype=np.float32)
    Wq = np.asarray(Wq, dtype=np.float32)
    Wk = np.asarray(Wk, dtype=np.float32)
    Wv = np.asarray(Wv, dtype=np.float32)
    Wo = np.asarray(Wo, dtype=np.float32)
    lq1 = np.asarray(lq1, dtype=np.float32)
    lk1 = np.asarray(lk1, dtype=np.float32)
    lq2 = np.asarray(lq2, dtype=np.float32)
    lk2 = np.asarray(lk2, dtype=np.float32)

    lam1 = np.exp(np.sum(lq1 * lk1, dtype=np.float32))
    lam2 = np.exp(np.sum(lq2 * lk2, dtype=np.float32))
    lam = float(lam1 - lam2 + LAMBDA_INIT)

    nc = _get_program(lam)
    in_maps = _prep_inputs(x, Wq, Wk, Wv, Wo)
    res = run_bass_kernel_spmd(nc, in_maps, core_ids=list(range(N_CORES)))

    out = np.empty((B, S, D), dtype=np.float32)
    for b in range(B):
        acc = res.results[4 * b]["outT"].astype(np.float32)
        for p in range(1, 4):
            acc += res.results[4 * b + p]["outT"].astype(np.float32)
        out[b] = acc.T
    return out
